# revision 30
# baseline (speedup 1.0000x reference)
"""Trainium2 Bass kernel for nn_AREConvFPN (vector-neuron GNN message passing).

Data-parallel over query points N=16000 across 8 NeuronCores (2000 pts/core).
Per-core layout: 16 tiles x 125 points-on-partitions; all per-point compute is
free-dim vector/scalar work using broadcast (step-0) access patterns; neighbor
gather of s_pts rows is done on-device with indirect DMA.

Self-contained: hardcodes all shapes; host side only slices/reshapes inputs.
"""

import math
import os
import numpy as np

DBG = os.environ.get("KDBG", "")

import concourse.bass as bass
import concourse.mybir as mybir
import concourse.tile as tile
from concourse.bass_utils import run_bass_kernel_spmd

F32 = mybir.dt.float32
I32 = mybir.dt.int32
ALU = mybir.AluOpType
ACTF = mybir.ActivationFunctionType
AXIS = mybir.AxisListType

NCORES = 8
N = 16000
N2 = 20000
K = 16
KS = 8          # kernel points
HALF = 32       # half of OUT
OUT = 64
NLOC = N // NCORES          # 2000 points per core
TP = 125                    # points per tile (partitions used)
NT = NLOC // TP             # 16 tiles
EPS = 1e-6
NEG = 0.2
BN = np.float32(1.0 / math.sqrt(1.0 + 1e-5))

_CACHE = {}


def _emit_vn_act(nc, pool, p_ap, d_ap, ch, nd, out_scale_blend, tag):
    """VN leaky relu on tensors laid out [TP, (ch, d=3) (+ maybe k)] with d at
    stride nd_inner. p_ap/d_ap are APs over [TP, ch*3*(extra)] tiles whose free
    layout is (ch, 3, inner) with inner size nd (nd=1 for no-k case).
    Returns a new tile AP [TP, ch*3*nd] holding the result.
    out_scale_blend: if True apply 0.2/0.8 blend; else caller folded 0.8 and we
    apply 0.25*p + sel (w/ 0.8 folded downstream).
    """
    FREE = ch * 3 * nd
    CF = ch * nd

    def v3(ap):  # [TP, ch, 3, nd]
        return ap.rearrange("p (c d i) -> p c d i", c=ch, d=3, i=nd)

    def v_red(ap):  # [TP, ch, nd, 3]  (d innermost for reduce)
        return ap.rearrange("p (c d i) -> p c i d", c=ch, d=3, i=nd)

    pd = pool.tile([TP, FREE], F32, tag=f"{tag}_pd", bufs=1)
    nc.vector.tensor_tensor(out=pd[:], in0=p_ap, in1=d_ap, op=ALU.mult)
    dot = pool.tile([TP, CF], F32, tag=f"{tag}_dot")
    nc.vector.tensor_reduce(
        out=dot[:].rearrange("p (c i) -> p c i", c=ch, i=nd),
        in_=v_red(pd[:]), axis=AXIS.X, op=ALU.add)
    dd = pool.tile([TP, FREE], F32, tag=f"{tag}_pd", bufs=1)
    nc.vector.tensor_tensor(out=dd[:], in0=d_ap, in1=d_ap, op=ALU.mult)
    dsq = pool.tile([TP, CF], F32, tag=f"{tag}_dsq")
    nc.vector.tensor_reduce(
        out=dsq[:].rearrange("p (c i) -> p c i", c=ch, i=nd),
        in_=v_red(dd[:]), axis=AXIS.X, op=ALU.add)
    den = pool.tile([TP, CF], F32, tag=f"{tag}_den")
    nc.vector.tensor_scalar(out=den[:], in0=dsq[:], scalar1=EPS, scalar2=None,
                            op0=ALU.add)
    rcp = pool.tile([TP, CF], F32, tag=f"{tag}_rcp")
    nc.vector.reciprocal(out=rcp[:], in_=den[:])
    rat = pool.tile([TP, CF], F32, tag=f"{tag}_rat")
    nc.vector.tensor_tensor(out=rat[:], in0=dot[:], in1=rcp[:], op=ALU.mult)

    # corr = p - rat*d
    tmp = pool.tile([TP, FREE], F32, tag=f"{tag}_pd", bufs=1)
    rat_b = (rat[:].rearrange("p (c i) -> p c i", c=ch, i=nd)
             .unsqueeze(2).to_broadcast([TP, ch, 3, nd]))
    nc.vector.tensor_tensor(out=v3(tmp[:]), in0=rat_b, in1=v3(d_ap), op=ALU.mult)
    corr = pool.tile([TP, FREE], F32, tag=f"{tag}_corr", bufs=1)
    nc.vector.tensor_tensor(out=corr[:], in0=p_ap, in1=tmp[:], op=ALU.subtract)

    mask = pool.tile([TP, FREE], mybir.dt.uint8, tag=f"{tag}_mask")
    dot_b = (dot[:].rearrange("p (c i) -> p c i", c=ch, i=nd)
             .unsqueeze(2).to_broadcast([TP, ch, 3, nd]))
    nc.vector.tensor_scalar(out=v3(mask[:]), in0=dot_b, scalar1=0.0,
                            scalar2=None, op0=ALU.is_ge)
    sel = pool.tile([TP, FREE], F32, tag=f"{tag}_sel", bufs=1)
    nc.vector.tensor_copy(out=sel[:], in_=corr[:])
    nc.vector.copy_predicated(out=sel[:], mask=mask[:], data=p_ap)

    res = pool.tile([TP, FREE], F32, tag=f"{tag}_res", bufs=2)
    if out_scale_blend:
        # res = 0.2*p + 0.8*sel
        t1 = pool.tile([TP, FREE], F32, tag=f"{tag}_pd", bufs=1)
        nc.scalar.activation(out=t1[:], in_=p_ap, func=ACTF.Copy, scale=float(NEG))
        t2 = pool.tile([TP, FREE], F32, tag=f"{tag}_corr", bufs=1)
        nc.scalar.activation(out=t2[:], in_=sel[:], func=ACTF.Copy,
                             scale=float(1.0 - NEG))
        nc.vector.tensor_tensor(out=res[:], in0=t1[:], in1=t2[:], op=ALU.add)
    else:
        # res = 0.25*p + sel   (0.8 folded into the next weights)
        t1 = pool.tile([TP, FREE], F32, tag=f"{tag}_pd", bufs=1)
        nc.scalar.activation(out=t1[:], in_=p_ap, func=ACTF.Copy,
                             scale=float(NEG / (1.0 - NEG)))
        nc.vector.tensor_tensor(out=res[:], in0=t1[:], in1=sel[:], op=ALU.add)
    return res


def _split_multi_waits(nc):
    """TRN2 compute/DMA instructions encode at most one semaphore wait.
    Tile sometimes emits several; hoist the extras onto standalone
    EventSemaphore instructions placed just before (same engine stream)."""
    n = 0
    for f in nc.m.functions:
        for blk in f.blocks:
            out = []
            changed = False
            for inst in blk.instructions:
                si = inst.sync_info
                if si is not None and si.on_wait and len(si.on_wait) > 1:
                    waits = list(si.on_wait)
                    for w in waits[:-1]:
                        ev = mybir.InstEventSemaphore(
                            name=f"sw-{n}-{inst.name}", engine=inst.engine,
                            ins=[], outs=[],
                            sync_info=mybir.SyncInfo(on_wait=[w],
                                                     on_update=[]))
                        out.append(ev)
                        n += 1
                    inst.sync_info = mybir.SyncInfo(
                        on_wait=[waits[-1]], on_update=list(si.on_update))
                    changed = True
                out.append(inst)
            if changed:
                blk.instructions = out
    return n


def _build_bass(split_waits=True):
    nc = bass.Bass()

    sp = nc.declare_dram_parameter("sp", [N2, 3], F32, isOutput=False)
    qa = nc.declare_dram_parameter("qa", [TP, NT * 3], F32, isOutput=False)
    ia = nc.declare_dram_parameter("ia", [TP, NT * K], I32, isOutput=False)
    w_shapes = dict(wvn=48, wdv=48, wh1=128, wh2=64, bh2=8, wbr=768,
                    wdr=1024, wun=2048, wdu=2048)
    wparams = {k: nc.declare_dram_parameter(k, [128, v], F32, isOutput=False)
               for k, v in w_shapes.items()}
    out = nc.declare_dram_parameter("out", [NLOC, OUT * 3], F32, isOutput=True)

    with tile.TileContext(nc) as tc:
        with (
            tc.tile_pool(name="wts", bufs=1) as wpool,
            tc.tile_pool(name="g", bufs=NT) as gpool,
            tc.tile_pool(name="work", bufs=2) as pool,
            tc.tile_pool(name="jumbo", bufs=1) as jpool,
        ):
            wsb = {}
            for k_, v in w_shapes.items():
                wsb[k_] = wpool.tile([128, v], F32, tag=f"w_{k_}",
                                     name=f"w_{k_}")
                nc.sync.dma_start(out=wsb[k_][:], in_=wparams[k_][:, :])
            qsb = wpool.tile([TP, NT * 3], F32, tag="qsb")
            nc.sync.dma_start(out=qsb[:], in_=qa[:, :])
            isb = wpool.tile([TP, NT * K], I32, tag="isb")
            nc.sync.dma_start(out=isb[:], in_=ia[:, :])

            # "touch" every staged tile once per consumer engine so later
            # instructions never need more than one DMA-sem wait each
            # (TRN2 allows a single sync-wait per compute instruction).
            dummy = wpool.tile([128, 16], F32, tag="dummy")
            dummyi = wpool.tile([128, 2], I32, tag="dummyi")
            for i, k_ in enumerate(w_shapes):
                nc.vector.tensor_scalar(out=dummy[:, i:i + 1],
                                        in0=wsb[k_][:, :1], scalar1=1.0,
                                        scalar2=None, op0=ALU.mult)
            nc.vector.tensor_scalar(out=dummy[:TP, 10:11], in0=qsb[:, :1],
                                    scalar1=1.0, scalar2=None, op0=ALU.mult)
            nc.gpsimd.tensor_scalar(out=dummyi[:TP, :1], in0=isb[:, :1],
                                    scalar1=1, scalar2=None, op0=ALU.mult)

            for t in range(NT):
                # ---- gather neighbors: G[p, k*3+d] = s_pts[idx[p,k], d]
                # HW indirect DMA uses one index per partition, so issue one
                # DMA per neighbor slot k.
                G = gpool.tile([TP, K * 3], F32, tag="G")
                for k in range(K):
                    nc.gpsimd.indirect_dma_start(
                        out=G[:, k * 3:(k + 1) * 3], out_offset=None,
                        in_=sp[:, :],
                        in_offset=bass.IndirectOffsetOnAxis(
                            ap=isb[:, t * K + k:t * K + k + 1], axis=0))

                # ---- pts = G - q  (broadcast q over k)
                pts = pool.tile([TP, K * 3], F32, tag="pts")
                q_b = (qsb[:, t * 3:(t + 1) * 3].unsqueeze(1)
                       .to_broadcast([TP, K, 3]))
                nc.vector.tensor_tensor(
                    out=pts[:].rearrange("p (k d) -> p k d", k=K, d=3),
                    in0=G[:].rearrange("p (k d) -> p k d", k=K, d=3),
                    in1=q_b, op=ALU.subtract)

                def pts_kd():
                    return pts[:].rearrange("p (k d) -> p k d", k=K, d=3)

                def pts_dk():
                    return pts[:].rearrange("p (k d) -> p d k", k=K, d=3)

                # ---- centers = mean_k pts  [TP, 3]
                cen = pool.tile([TP, 3], F32, tag="cen")
                nc.vector.tensor_reduce(out=cen[:], in_=pts_dk(),
                                        axis=AXIS.X, op=ALU.add)
                nc.vector.tensor_scalar(out=cen[:], in0=cen[:],
                                        scalar1=1.0 / K, scalar2=None,
                                        op0=ALU.mult)

                # ---- cross = pts x cen  [TP, (k,d)]
                cross = pool.tile([TP, K * 3], F32, tag="cross")
                crv = cross[:].rearrange("p (k d) -> p k d", k=K, d=3)
                ct1 = pool.tile([TP, K], F32, tag="ct1")
                ct2 = pool.tile([TP, K], F32, tag="ct2")
                for dd in range(3):
                    d1, d2 = (dd + 1) % 3, (dd + 2) % 3
                    nc.vector.tensor_tensor(
                        out=ct1[:], in0=pts_kd()[:, :, d1],
                        in1=cen[:, d2:d2 + 1].to_broadcast([TP, K]),
                        op=ALU.mult)
                    nc.vector.tensor_tensor(
                        out=ct2[:], in0=pts_kd()[:, :, d2],
                        in1=cen[:, d1:d1 + 1].to_broadcast([TP, K]),
                        op=ALU.mult)
                    nc.vector.tensor_tensor(
                        out=crv[:, :, dd], in0=ct1[:], in1=ct2[:],
                        op=ALU.subtract)

                if DBG == "local":
                    nc.sync.dma_start(out=out[t * TP:(t + 1) * TP, :48],
                                      in_=pts[:])
                    nc.sync.dma_start(out=out[t * TP:(t + 1) * TP, 48:96],
                                      in_=cross[:])
                    nc.sync.dma_start(out=out[t * TP:(t + 1) * TP, 96:99],
                                      in_=cen[:])
                    nc.sync.dma_start(out=out[t * TP:(t + 1) * TP, 99:147],
                                      in_=G[:])
                    continue

                def cross_dk():
                    return cross[:].rearrange("p (k d) -> p d k", k=K, d=3)

                def cen_b(o):  # [TP, o, 3, K] broadcast
                    return (cen[:].unsqueeze(1).unsqueeze(3)
                            .to_broadcast([TP, o, 3, K]))

                def loc_b(ap, o):  # local (k,d) tile -> [TP, o, 3, K]
                    return (ap.rearrange("p (k d) -> p d k", k=K, d=3)
                            .unsqueeze(1).to_broadcast([TP, o, 3, K]))

                # ---- p = vn_lin(local, wvn) (BN folded), layout (o, d, k)
                def emit_vnlin(wtile, o, dst_tag):
                    dst = pool.tile([TP, o * 3 * K], F32, tag=dst_tag)
                    dv = dst[:].rearrange("p (o d k) -> p o d k", o=o, d=3, k=K)
                    wv = wtile[:TP, :].rearrange("p (c o) -> p c o", c=3, o=o)
                    tmp = pool.tile([TP, o * 3 * K], F32, tag="vl_tmp", bufs=1)
                    tv = tmp[:].rearrange("p (o d k) -> p o d k", o=o, d=3, k=K)
                    for c, src in enumerate((loc_b(pts[:], o), cen_b(o),
                                             loc_b(cross[:], o))):
                        wb_ = (wv[:, c].unsqueeze(2).unsqueeze(3)
                               .to_broadcast([TP, o, 3, K]))
                        if c == 0:
                            nc.vector.tensor_tensor(out=dv, in0=src, in1=wb_,
                                                    op=ALU.mult)
                        else:
                            nc.vector.tensor_tensor(out=tv, in0=src, in1=wb_,
                                                    op=ALU.mult)
                            nc.vector.tensor_tensor(out=dv, in0=dv, in1=tv,
                                                    op=ALU.add)
                    return dst

                P_ = emit_vnlin(wsb["wvn"], 2 * KS, "P_")
                D_ = emit_vnlin(wsb["wdv"], 2 * KS, "D_")

                # ---- s = vn_act(p, d) with 0.8 folded into wh1
                S_ = _emit_vn_act(nc, pool, P_[:], D_[:], 2 * KS, K,
                                  out_scale_blend=False, tag="va1")

                # ---- snorm = ||s||_d  [TP, (o,k)]
                ss = pool.tile([TP, 2 * KS * 3 * K], F32, tag="va1_pd",
                               bufs=1)
                nc.vector.tensor_tensor(out=ss[:], in0=S_[:], in1=S_[:],
                                        op=ALU.mult)
                nsq = pool.tile([TP, 2 * KS * K], F32, tag="nsq")
                nc.vector.tensor_reduce(
                    out=nsq[:].rearrange("p (o k) -> p o k", o=2 * KS, k=K),
                    in_=ss[:].rearrange("p (o d k) -> p o k d",
                                        o=2 * KS, d=3, k=K),
                    axis=AXIS.X, op=ALU.add)
                sn = pool.tile([TP, 2 * KS * K], F32, tag="sn")
                nc.scalar.activation(out=sn[:], in_=nsq[:], func=ACTF.Sqrt)

                # ---- h1: z1[o2,k] = relu(sum_o sn[o,k]*wh1[o,o2])
                tw = jpool.tile([TP, 6144], F32, tag="jumbo")
                twv = (tw[:, :KS * K * 2 * KS]
                       .rearrange("p (o2 k o) -> p o2 k o",
                                  o2=KS, k=K, o=2 * KS))
                sn_b = (sn[:].rearrange("p (o k) -> p k o", o=2 * KS, k=K)
                        .unsqueeze(1).to_broadcast([TP, KS, K, 2 * KS]))
                wh1_b = (wsb["wh1"][:TP, :]
                         .rearrange("p (o o2) -> p o2 o", o=2 * KS, o2=KS)
                         .unsqueeze(2).to_broadcast([TP, KS, K, 2 * KS]))
                nc.vector.tensor_tensor(out=twv, in0=sn_b, in1=wh1_b,
                                        op=ALU.mult)
                z1 = pool.tile([TP, KS * K], F32, tag="z1")
                nc.vector.tensor_reduce(
                    out=z1[:].rearrange("p (o2 k) -> p o2 k", o2=KS, k=K),
                    in_=twv, axis=AXIS.X, op=ALU.add)
                z1r = pool.tile([TP, KS * K], F32, tag="z1r")
                nc.scalar.activation(out=z1r[:], in_=z1[:], func=ACTF.Relu)

                # ---- h2 + bias
                tw2v = (tw[:, :KS * K * KS]
                        .rearrange("p (o3 k o2) -> p o3 k o2",
                                   o3=KS, k=K, o2=KS))
                z1_b = (z1r[:].rearrange("p (o2 k) -> p k o2", o2=KS, k=K)
                        .unsqueeze(1).to_broadcast([TP, KS, K, KS]))
                wh2_b = (wsb["wh2"][:TP, :]
                         .rearrange("p (o2 o3) -> p o3 o2", o2=KS, o3=KS)
                         .unsqueeze(2).to_broadcast([TP, KS, K, KS]))
                nc.vector.tensor_tensor(out=tw2v, in0=z1_b, in1=wh2_b,
                                        op=ALU.mult)
                z2 = pool.tile([TP, KS * K], F32, tag="z2")
                nc.vector.tensor_reduce(
                    out=z2[:].rearrange("p (o3 k) -> p o3 k", o3=KS, k=K),
                    in_=tw2v, axis=AXIS.X, op=ALU.add)
                bh2_b = (wsb["bh2"][:TP, :].unsqueeze(2)
                         .to_broadcast([TP, KS, K]))
                z2v = z2[:].rearrange("p (o3 k) -> p o3 k", o3=KS, k=K)
                nc.vector.tensor_tensor(out=z2v, in0=z2v, in1=bh2_b,
                                        op=ALU.add)

                # ---- softmax over o3
                mx = pool.tile([TP, K], F32, tag="mx")
                nc.vector.tensor_reduce(
                    out=mx[:],
                    in_=z2[:].rearrange("p (o3 k) -> p k o3", o3=KS, k=K),
                    axis=AXIS.X, op=ALU.max)
                mx_b = mx[:].unsqueeze(1).to_broadcast([TP, KS, K])
                ex = pool.tile([TP, KS * K], F32, tag="ex")
                exv = ex[:].rearrange("p (o3 k) -> p o3 k", o3=KS, k=K)
                nc.vector.tensor_tensor(out=exv, in0=z2v, in1=mx_b,
                                        op=ALU.subtract)
                exe = pool.tile([TP, KS * K], F32, tag="exe")
                nc.scalar.activation(out=exe[:], in_=ex[:], func=ACTF.Exp)
                exv = exe[:].rearrange("p (o3 k) -> p o3 k", o3=KS, k=K)
                sme = pool.tile([TP, K], F32, tag="sme")
                nc.vector.tensor_reduce(
                    out=sme[:],
                    in_=exe[:].rearrange("p (o3 k) -> p k o3", o3=KS, k=K),
                    axis=AXIS.X, op=ALU.add)
                rcs = pool.tile([TP, K], F32, tag="rcs")
                nc.vector.reciprocal(out=rcs[:], in_=sme[:])
                sc = pool.tile([TP, KS * K], F32, tag="sc")
                scv = sc[:].rearrange("p (ks k) -> p ks k", ks=KS, k=K)
                nc.vector.tensor_tensor(
                    out=scv, in0=exv,
                    in1=rcs[:].unsqueeze(1).to_broadcast([TP, KS, K]),
                    op=ALU.mult)

                # ---- weff[(c,h),k] = sum_ks sc[ks,k] * wbr[(c,h),ks]
                weff = pool.tile([TP, 96 * K], F32, tag="weff", bufs=1)
                KC = K // 2
                for kc in range(2):
                    tw3v = (tw[:, :96 * KC * KS]
                            .rearrange("p (ch k ks) -> p ch k ks",
                                       ch=96, k=KC, ks=KS))
                    sc_b = (sc[:].rearrange("p (ks k) -> p k ks", ks=KS, k=K)
                            [:, kc * KC:(kc + 1) * KC]
                            .unsqueeze(1).to_broadcast([TP, 96, KC, KS]))
                    wbr_b = (wsb["wbr"][:TP, :]
                             .rearrange("p (ch ks) -> p ch ks", ch=96, ks=KS)
                             .unsqueeze(2).to_broadcast([TP, 96, KC, KS]))
                    nc.vector.tensor_tensor(out=tw3v, in0=sc_b, in1=wbr_b,
                                            op=ALU.mult)
                    nc.vector.tensor_reduce(
                        out=weff[:].rearrange("p (ch k) -> p ch k",
                                              ch=96, k=K)
                        [:, :, kc * KC:(kc + 1) * KC],
                        in_=tw3v, axis=AXIS.X, op=ALU.add)

                # ---- pro[(h,d),k] = sum_c weff[(c,h),k] * local[c][d,k]
                pro = pool.tile([TP, HALF * 3 * K], F32, tag="pro", bufs=1)
                prov = pro[:].rearrange("p (h d k) -> p h d k",
                                        h=HALF, d=3, k=K)
                wev = weff[:].rearrange("p (c h k) -> p c h k",
                                        c=3, h=HALF, k=K)
                ptmp = pool.tile([TP, HALF * 3 * K], F32, tag="ptmp", bufs=1)
                ptv = ptmp[:].rearrange("p (h d k) -> p h d k",
                                        h=HALF, d=3, k=K)
                for c, src in enumerate((loc_b(pts[:], HALF), cen_b(HALF),
                                         loc_b(cross[:], HALF))):
                    we_b = wev[:, c].unsqueeze(2).to_broadcast(
                        [TP, HALF, 3, K])
                    if c == 0:
                        nc.vector.tensor_tensor(out=prov, in0=we_b, in1=src,
                                                op=ALU.mult)
                    else:
                        nc.vector.tensor_tensor(out=ptv, in0=we_b, in1=src,
                                                op=ALU.mult)
                        nc.vector.tensor_tensor(out=prov, in0=prov, in1=ptv,
                                                op=ALU.add)

                # ---- normalize over d, then mean over k -> feats [TP, (h,d)]
                q2 = pool.tile([TP, HALF * 3 * K], F32, tag="ptmp", bufs=1)
                nc.vector.tensor_tensor(out=q2[:], in0=pro[:], in1=pro[:],
                                        op=ALU.mult)
                ns = pool.tile([TP, HALF * K], F32, tag="ns")
                nc.vector.tensor_reduce(
                    out=ns[:].rearrange("p (h k) -> p h k", h=HALF, k=K),
                    in_=q2[:].rearrange("p (h d k) -> p h k d",
                                        h=HALF, d=3, k=K),
                    axis=AXIS.X, op=ALU.add)
                nr = pool.tile([TP, HALF * K], F32, tag="nr")
                nc.scalar.activation(out=nr[:], in_=ns[:], func=ACTF.Sqrt)
                nrc = pool.tile([TP, HALF * K], F32, tag="nrc")
                nc.vector.tensor_scalar(out=nrc[:], in0=nr[:], scalar1=1e-12,
                                        scalar2=None, op0=ALU.max)
                rcn = pool.tile([TP, HALF * K], F32, tag="rcn")
                nc.vector.reciprocal(out=rcn[:], in_=nrc[:])
                nmd = pool.tile([TP, HALF * 3 * K], F32, tag="nmd", bufs=1)
                rcn_b = (rcn[:].rearrange("p (h k) -> p h k", h=HALF, k=K)
                         .unsqueeze(2).to_broadcast([TP, HALF, 3, K]))
                nc.vector.tensor_tensor(
                    out=nmd[:].rearrange("p (h d k) -> p h d k",
                                         h=HALF, d=3, k=K),
                    in0=prov, in1=rcn_b, op=ALU.mult)
                fe = pool.tile([TP, HALF * 3], F32, tag="fe")
                nc.vector.tensor_reduce(
                    out=fe[:].rearrange("p (h d) -> p h d", h=HALF, d=3),
                    in_=nmd[:].rearrange("p (h d k) -> p h d k",
                                         h=HALF, d=3, k=K),
                    axis=AXIS.X, op=ALU.add)
                nc.vector.tensor_scalar(out=fe[:], in0=fe[:], scalar1=1.0 / K,
                                        scalar2=None, op0=ALU.mult)

                # ---- d1 = vn_lin(fe, wd_relu); f2 = vn_act(fe, d1)
                def emit_fc(src_tile, wtile, ci, co, dst_tag):
                    # dst[(o,d)] = sum_hi src[(hi,d)] * w[(o,hi)]
                    twv_ = (tw[:, :co * 3 * ci]
                            .rearrange("p (o d hi) -> p o d hi",
                                       o=co, d=3, hi=ci))
                    src_b = (src_tile[:].rearrange("p (h d) -> p d h",
                                                   h=ci, d=3)
                             .unsqueeze(1).to_broadcast([TP, co, 3, ci]))
                    w_b = (wtile[:TP, :co * ci]
                           .rearrange("p (o hi) -> p o hi", o=co, hi=ci)
                           .unsqueeze(2).to_broadcast([TP, co, 3, ci]))
                    nc.vector.tensor_tensor(out=twv_, in0=src_b, in1=w_b,
                                            op=ALU.mult)
                    dst = pool.tile([TP, co * 3], F32, tag=dst_tag)
                    nc.vector.tensor_reduce(
                        out=dst[:].rearrange("p (o d) -> p o d", o=co, d=3),
                        in_=twv_, axis=AXIS.X, op=ALU.add)
                    return dst

                d1 = emit_fc(fe, wsb["wdr"], HALF, HALF, "d1")
                f2 = _emit_vn_act(nc, pool, fe[:], d1[:], HALF, 1,
                                  out_scale_blend=True, tag="va2")

                # ---- out = vn_act(vn_lin(f2, w_un)*BN, vn_lin(f2, wd_un))
                u = emit_fc(f2, wsb["wun"], HALF, OUT, "u")
                du = emit_fc(f2, wsb["wdu"], HALF, OUT, "du")
                ot = _emit_vn_act(nc, pool, u[:], du[:], OUT, 1,
                                  out_scale_blend=True, tag="va3")

                nc.sync.dma_start(out=out[t * TP:(t + 1) * TP, :], in_=ot[:])

    if split_waits:
        _split_multi_waits(nc)
    return nc


def _prep_core_inputs(core, q_pts, s_pts, idx, weights):
    q = q_pts[core * NLOC:(core + 1) * NLOC].astype(np.float32)
    ix = idx[core * NLOC:(core + 1) * NLOC].astype(np.int32)
    qa = np.ascontiguousarray(
        q.reshape(NT, TP, 3).transpose(1, 0, 2).reshape(TP, NT * 3))
    ia = np.ascontiguousarray(
        ix.reshape(NT, TP, K).transpose(1, 0, 2).reshape(TP, NT * K))
    m = {"sp": np.ascontiguousarray(s_pts.astype(np.float32)),
         "qa": qa, "ia": ia}
    m.update(weights)
    return m


def kernel(q_pts, s_pts, s_feats, neighbor_indices, wb, w_vn, wd_vn,
           w_h1, w_h2, b_h2, wd_relu, w_un, wd_un):
    q_pts = np.asarray(q_pts, dtype=np.float32)
    s_pts = np.asarray(s_pts, dtype=np.float32)
    idx = np.asarray(neighbor_indices)
    wb = np.asarray(wb, np.float32); w_vn = np.asarray(w_vn, np.float32)
    wd_vn = np.asarray(wd_vn, np.float32); w_h1 = np.asarray(w_h1, np.float32)
    w_h2 = np.asarray(w_h2, np.float32); b_h2 = np.asarray(b_h2, np.float32)
    wd_relu = np.asarray(wd_relu, np.float32)
    w_un = np.asarray(w_un, np.float32); wd_un = np.asarray(wd_un, np.float32)

    def rep(a):
        a = np.ascontiguousarray(a.reshape(1, -1), dtype=np.float32)
        return np.ascontiguousarray(np.repeat(a, 128, axis=0))

    # host-side weight packing (layouts documented at each use site)
    weights = {
        "wvn": rep((w_vn * BN)),                                  # (c,o)
        "wdv": rep(wd_vn),                                        # (c,o)
        "wh1": rep(w_h1 * BN * (1.0 - NEG)),                      # (o,o2)
        "wh2": rep(w_h2),                                         # (o2,o3)
        "bh2": rep(b_h2),                                         # (o3,)
        # wbr[(c,h),ks] = wb[c, ks*HALF+h]
        "wbr": rep(wb.reshape(3, KS, HALF).transpose(0, 2, 1)),   # (c,h,ks)
        # wdr[(ho,hi)] = wd_relu[hi,ho]
        "wdr": rep(wd_relu.T),
        "wun": rep((w_un * BN).T),                                # (o,hi)
        "wdu": rep(wd_un.T),                                      # (o,hi)
    }

    if "nc" not in _CACHE:
        _CACHE["nc"] = _build_bass()
    nc = _CACHE["nc"]

    in_maps = [_prep_core_inputs(c, q_pts, s_pts, idx, weights)
               for c in range(NCORES)]
    res = run_bass_kernel_spmd(nc, in_maps, core_ids=list(range(NCORES)))
    outs = [np.asarray(r["out"]).reshape(NLOC, OUT, 3) for r in res.results]
    return np.concatenate(outs, axis=0)


# revision 34
# speedup vs baseline: 1.1256x; 1.1256x over previous
"""Trainium2 Bass kernel for nn_AREConvFPN (vector-neuron GNN message passing).

Data-parallel over query points N=16000 across 8 NeuronCores (2000 pts/core).
Per-core layout: 16 tiles x 125 points-on-partitions; all per-point compute is
free-dim vector/scalar work using broadcast (step-0) access patterns; neighbor
gather of s_pts rows is done on-device with indirect DMA.

Self-contained: hardcodes all shapes; host side only slices/reshapes inputs.
"""

import math
import os
import numpy as np

DBG = os.environ.get("KDBG", "")

import concourse.bass as bass
import concourse.mybir as mybir
import concourse.tile as tile
from concourse.bass_utils import run_bass_kernel_spmd

F32 = mybir.dt.float32
I32 = mybir.dt.int32
ALU = mybir.AluOpType
ACTF = mybir.ActivationFunctionType
AXIS = mybir.AxisListType

NCORES = 8
N = 16000
N2 = 20000
K = 16
KS = 8          # kernel points
HALF = 32       # half of OUT
OUT = 64
NLOC = N // NCORES          # 2000 points per core
TP = 125                    # points per tile (partitions used)
NT = NLOC // TP             # 16 tiles
EPS = 1e-6
NEG = 0.2
BN = np.float32(1.0 / math.sqrt(1.0 + 1e-5))

_CACHE = {}


def _emit_vn_act(nc, pool, p_ap, d_ap, ch, nd, out_scale_blend, tag):
    """VN leaky relu on tensors laid out [TP, (ch, d=3) (+ maybe k)] with d at
    stride nd_inner. p_ap/d_ap are APs over [TP, ch*3*(extra)] tiles whose free
    layout is (ch, 3, inner) with inner size nd (nd=1 for no-k case).
    Returns a new tile AP [TP, ch*3*nd] holding the result.
    out_scale_blend: if True apply 0.2/0.8 blend; else caller folded 0.8 and we
    apply 0.25*p + sel (w/ 0.8 folded downstream).
    """
    FREE = ch * 3 * nd
    CF = ch * nd

    def v3(ap):  # [TP, ch, 3, nd]
        return ap.rearrange("p (c d i) -> p c d i", c=ch, d=3, i=nd)

    def v_red(ap):  # [TP, ch, nd, 3]  (d innermost for reduce)
        return ap.rearrange("p (c d i) -> p c i d", c=ch, d=3, i=nd)

    pd = pool.tile([TP, FREE], F32, tag=f"{tag}_pd", bufs=1)
    nc.vector.tensor_tensor(out=pd[:], in0=p_ap, in1=d_ap, op=ALU.mult)
    dot = pool.tile([TP, CF], F32, tag=f"{tag}_dot")
    nc.vector.tensor_reduce(
        out=dot[:].rearrange("p (c i) -> p c i", c=ch, i=nd),
        in_=v_red(pd[:]), axis=AXIS.X, op=ALU.add)
    dd = pool.tile([TP, FREE], F32, tag=f"{tag}_pd", bufs=1)
    nc.vector.tensor_tensor(out=dd[:], in0=d_ap, in1=d_ap, op=ALU.mult)
    dsq = pool.tile([TP, CF], F32, tag=f"{tag}_dsq")
    nc.vector.tensor_reduce(
        out=dsq[:].rearrange("p (c i) -> p c i", c=ch, i=nd),
        in_=v_red(dd[:]), axis=AXIS.X, op=ALU.add)
    den = pool.tile([TP, CF], F32, tag=f"{tag}_den")
    nc.vector.tensor_scalar(out=den[:], in0=dsq[:], scalar1=EPS, scalar2=None,
                            op0=ALU.add)
    rcp = pool.tile([TP, CF], F32, tag=f"{tag}_rcp")
    nc.vector.reciprocal(out=rcp[:], in_=den[:])
    rat = pool.tile([TP, CF], F32, tag=f"{tag}_rat")
    nc.vector.tensor_tensor(out=rat[:], in0=dot[:], in1=rcp[:], op=ALU.mult)

    # corr = p - rat*d
    tmp = pool.tile([TP, FREE], F32, tag=f"{tag}_pd", bufs=1)
    rat_b = (rat[:].rearrange("p (c i) -> p c i", c=ch, i=nd)
             .unsqueeze(2).to_broadcast([TP, ch, 3, nd]))
    nc.vector.tensor_tensor(out=v3(tmp[:]), in0=rat_b, in1=v3(d_ap), op=ALU.mult)
    corr = pool.tile([TP, FREE], F32, tag=f"{tag}_corr", bufs=1)
    nc.vector.tensor_tensor(out=corr[:], in0=p_ap, in1=tmp[:], op=ALU.subtract)

    mask = pool.tile([TP, FREE], mybir.dt.uint8, tag=f"{tag}_mask")
    dot_b = (dot[:].rearrange("p (c i) -> p c i", c=ch, i=nd)
             .unsqueeze(2).to_broadcast([TP, ch, 3, nd]))
    nc.vector.tensor_scalar(out=v3(mask[:]), in0=dot_b, scalar1=0.0,
                            scalar2=None, op0=ALU.is_ge)
    sel = pool.tile([TP, FREE], F32, tag=f"{tag}_sel", bufs=1)
    nc.vector.tensor_copy(out=sel[:], in_=corr[:])
    nc.vector.copy_predicated(out=sel[:], mask=mask[:], data=p_ap)

    res = pool.tile([TP, FREE], F32, tag=f"{tag}_res", bufs=2)
    if out_scale_blend:
        # res = 0.2*p + 0.8*sel
        t1 = pool.tile([TP, FREE], F32, tag=f"{tag}_pd", bufs=1)
        nc.scalar.activation(out=t1[:], in_=p_ap, func=ACTF.Copy, scale=float(NEG))
        t2 = pool.tile([TP, FREE], F32, tag=f"{tag}_corr", bufs=1)
        nc.scalar.activation(out=t2[:], in_=sel[:], func=ACTF.Copy,
                             scale=float(1.0 - NEG))
        nc.vector.tensor_tensor(out=res[:], in0=t1[:], in1=t2[:], op=ALU.add)
    else:
        # res = 0.25*p + sel   (0.8 folded into the next weights)
        t1 = pool.tile([TP, FREE], F32, tag=f"{tag}_pd", bufs=1)
        nc.scalar.activation(out=t1[:], in_=p_ap, func=ACTF.Copy,
                             scale=float(NEG / (1.0 - NEG)))
        nc.vector.tensor_tensor(out=res[:], in0=t1[:], in1=sel[:], op=ALU.add)
    return res


def _split_multi_waits(nc):
    """TRN2 compute/DMA instructions encode at most one semaphore wait.
    Tile sometimes emits several; hoist the extras onto standalone
    EventSemaphore instructions placed just before (same engine stream)."""
    n = 0
    for f in nc.m.functions:
        for blk in f.blocks:
            out = []
            changed = False
            for inst in blk.instructions:
                si = inst.sync_info
                if si is not None and si.on_wait and len(si.on_wait) > 1:
                    waits = list(si.on_wait)
                    for w in waits[:-1]:
                        ev = mybir.InstEventSemaphore(
                            name=f"sw-{n}-{inst.name}", engine=inst.engine,
                            ins=[], outs=[],
                            sync_info=mybir.SyncInfo(on_wait=[w],
                                                     on_update=[]))
                        out.append(ev)
                        n += 1
                    inst.sync_info = mybir.SyncInfo(
                        on_wait=[waits[-1]], on_update=list(si.on_update))
                    changed = True
                out.append(inst)
            if changed:
                blk.instructions = out
    return n


def _build_bass(split_waits=True):
    nc = bass.Bass()

    sp = nc.declare_dram_parameter("sp", [N2, 3], F32, isOutput=False)
    qa = nc.declare_dram_parameter("qa", [TP, NT * 3], F32, isOutput=False)
    ia = nc.declare_dram_parameter("ia", [TP, NT * K], I32, isOutput=False)
    w_shapes = dict(wvn=48, wdv=48, wh1=128, wh2=64, bh2=8, wbr=768,
                    wdr=1024, wun=2048, wdu=2048)
    wparams = {k: nc.declare_dram_parameter(k, [1, v], F32, isOutput=False)
               for k, v in w_shapes.items()}
    out = nc.declare_dram_parameter("out", [NLOC, OUT * 3], F32, isOutput=True)

    with tile.TileContext(nc) as tc:
        with (
            tc.tile_pool(name="wts", bufs=1) as wpool,
            tc.tile_pool(name="g", bufs=NT) as gpool,
            tc.tile_pool(name="work", bufs=2) as pool,
            tc.tile_pool(name="jumbo", bufs=1) as jpool,
        ):
            wsb = {}
            for k_, v in w_shapes.items():
                wsb[k_] = wpool.tile([128, v], F32, tag=f"w_{k_}",
                                     name=f"w_{k_}")
                nc.sync.dma_start(
                    out=wsb[k_][:],
                    in_=wparams[k_][:, :].to_broadcast([128, v]))
            qsb = wpool.tile([TP, NT * 3], F32, tag="qsb")
            nc.sync.dma_start(out=qsb[:], in_=qa[:, :])
            isb = wpool.tile([TP, NT * K], I32, tag="isb")
            nc.sync.dma_start(out=isb[:], in_=ia[:, :])

            # "touch" every staged tile once per consumer engine so later
            # instructions never need more than one DMA-sem wait each
            # (TRN2 allows a single sync-wait per compute instruction).
            dummy = wpool.tile([128, 16], F32, tag="dummy")
            dummyi = wpool.tile([128, 2], I32, tag="dummyi")
            for i, k_ in enumerate(w_shapes):
                nc.vector.tensor_scalar(out=dummy[:, i:i + 1],
                                        in0=wsb[k_][:, :1], scalar1=1.0,
                                        scalar2=None, op0=ALU.mult)
            nc.vector.tensor_scalar(out=dummy[:TP, 10:11], in0=qsb[:, :1],
                                    scalar1=1.0, scalar2=None, op0=ALU.mult)
            nc.gpsimd.tensor_scalar(out=dummyi[:TP, :1], in0=isb[:, :1],
                                    scalar1=1, scalar2=None, op0=ALU.mult)

            for t in range(NT):
                # ---- gather neighbors: G[p, k*3+d] = s_pts[idx[p,k], d]
                # HW indirect DMA uses one index per partition, so issue one
                # DMA per neighbor slot k.
                G = gpool.tile([TP, K * 3], F32, tag="G")
                for k in range(K):
                    nc.gpsimd.indirect_dma_start(
                        out=G[:, k * 3:(k + 1) * 3], out_offset=None,
                        in_=sp[:, :],
                        in_offset=bass.IndirectOffsetOnAxis(
                            ap=isb[:, t * K + k:t * K + k + 1], axis=0))

                # ---- pts = G - q  (broadcast q over k)
                pts = pool.tile([TP, K * 3], F32, tag="pts")
                q_b = (qsb[:, t * 3:(t + 1) * 3].unsqueeze(1)
                       .to_broadcast([TP, K, 3]))
                nc.vector.tensor_tensor(
                    out=pts[:].rearrange("p (k d) -> p k d", k=K, d=3),
                    in0=G[:].rearrange("p (k d) -> p k d", k=K, d=3),
                    in1=q_b, op=ALU.subtract)

                def pts_kd():
                    return pts[:].rearrange("p (k d) -> p k d", k=K, d=3)

                def pts_dk():
                    return pts[:].rearrange("p (k d) -> p d k", k=K, d=3)

                # ---- centers = mean_k pts  [TP, 3]
                cen = pool.tile([TP, 3], F32, tag="cen")
                nc.vector.tensor_reduce(out=cen[:], in_=pts_dk(),
                                        axis=AXIS.X, op=ALU.add)
                nc.vector.tensor_scalar(out=cen[:], in0=cen[:],
                                        scalar1=1.0 / K, scalar2=None,
                                        op0=ALU.mult)

                # ---- cross = pts x cen  [TP, (k,d)]
                cross = pool.tile([TP, K * 3], F32, tag="cross")
                crv = cross[:].rearrange("p (k d) -> p k d", k=K, d=3)
                ct1 = pool.tile([TP, K], F32, tag="ct1")
                ct2 = pool.tile([TP, K], F32, tag="ct2")
                for dd in range(3):
                    d1, d2 = (dd + 1) % 3, (dd + 2) % 3
                    nc.vector.tensor_tensor(
                        out=ct1[:], in0=pts_kd()[:, :, d1],
                        in1=cen[:, d2:d2 + 1].to_broadcast([TP, K]),
                        op=ALU.mult)
                    nc.vector.tensor_tensor(
                        out=ct2[:], in0=pts_kd()[:, :, d2],
                        in1=cen[:, d1:d1 + 1].to_broadcast([TP, K]),
                        op=ALU.mult)
                    nc.vector.tensor_tensor(
                        out=crv[:, :, dd], in0=ct1[:], in1=ct2[:],
                        op=ALU.subtract)

                if DBG == "local":
                    nc.sync.dma_start(out=out[t * TP:(t + 1) * TP, :48],
                                      in_=pts[:])
                    nc.sync.dma_start(out=out[t * TP:(t + 1) * TP, 48:96],
                                      in_=cross[:])
                    nc.sync.dma_start(out=out[t * TP:(t + 1) * TP, 96:99],
                                      in_=cen[:])
                    nc.sync.dma_start(out=out[t * TP:(t + 1) * TP, 99:147],
                                      in_=G[:])
                    continue

                def cross_dk():
                    return cross[:].rearrange("p (k d) -> p d k", k=K, d=3)

                def cen_b(o):  # [TP, o, 3, K] broadcast
                    return (cen[:].unsqueeze(1).unsqueeze(3)
                            .to_broadcast([TP, o, 3, K]))

                def loc_b(ap, o):  # local (k,d) tile -> [TP, o, 3, K]
                    return (ap.rearrange("p (k d) -> p d k", k=K, d=3)
                            .unsqueeze(1).to_broadcast([TP, o, 3, K]))

                # ---- p = vn_lin(local, wvn) (BN folded), layout (o, d, k)
                def emit_vnlin(wtile, o, dst_tag):
                    dst = pool.tile([TP, o * 3 * K], F32, tag=dst_tag)
                    dv = dst[:].rearrange("p (o d k) -> p o d k", o=o, d=3, k=K)
                    wv = wtile[:TP, :].rearrange("p (c o) -> p c o", c=3, o=o)
                    tmp = pool.tile([TP, o * 3 * K], F32, tag="vl_tmp", bufs=1)
                    tv = tmp[:].rearrange("p (o d k) -> p o d k", o=o, d=3, k=K)
                    for c, src in enumerate((loc_b(pts[:], o), cen_b(o),
                                             loc_b(cross[:], o))):
                        wb_ = (wv[:, c].unsqueeze(2).unsqueeze(3)
                               .to_broadcast([TP, o, 3, K]))
                        if c == 0:
                            nc.vector.tensor_tensor(out=dv, in0=src, in1=wb_,
                                                    op=ALU.mult)
                        else:
                            nc.vector.tensor_tensor(out=tv, in0=src, in1=wb_,
                                                    op=ALU.mult)
                            nc.vector.tensor_tensor(out=dv, in0=dv, in1=tv,
                                                    op=ALU.add)
                    return dst

                P_ = emit_vnlin(wsb["wvn"], 2 * KS, "P_")
                D_ = emit_vnlin(wsb["wdv"], 2 * KS, "D_")

                # ---- s = vn_act(p, d) with 0.8 folded into wh1
                S_ = _emit_vn_act(nc, pool, P_[:], D_[:], 2 * KS, K,
                                  out_scale_blend=False, tag="va1")

                # ---- snorm = ||s||_d  [TP, (o,k)]
                ss = pool.tile([TP, 2 * KS * 3 * K], F32, tag="va1_pd",
                               bufs=1)
                nc.vector.tensor_tensor(out=ss[:], in0=S_[:], in1=S_[:],
                                        op=ALU.mult)
                nsq = pool.tile([TP, 2 * KS * K], F32, tag="nsq")
                nc.vector.tensor_reduce(
                    out=nsq[:].rearrange("p (o k) -> p o k", o=2 * KS, k=K),
                    in_=ss[:].rearrange("p (o d k) -> p o k d",
                                        o=2 * KS, d=3, k=K),
                    axis=AXIS.X, op=ALU.add)
                sn = pool.tile([TP, 2 * KS * K], F32, tag="sn")
                nc.scalar.activation(out=sn[:], in_=nsq[:], func=ACTF.Sqrt)

                # ---- h1: z1[o2,k] = relu(sum_o sn[o,k]*wh1[o,o2])
                tw = jpool.tile([TP, 6144], F32, tag="jumbo")
                twv = (tw[:, :KS * K * 2 * KS]
                       .rearrange("p (o2 k o) -> p o2 k o",
                                  o2=KS, k=K, o=2 * KS))
                sn_b = (sn[:].rearrange("p (o k) -> p k o", o=2 * KS, k=K)
                        .unsqueeze(1).to_broadcast([TP, KS, K, 2 * KS]))
                wh1_b = (wsb["wh1"][:TP, :]
                         .rearrange("p (o o2) -> p o2 o", o=2 * KS, o2=KS)
                         .unsqueeze(2).to_broadcast([TP, KS, K, 2 * KS]))
                nc.vector.tensor_tensor(out=twv, in0=sn_b, in1=wh1_b,
                                        op=ALU.mult)
                z1 = pool.tile([TP, KS * K], F32, tag="z1")
                nc.vector.tensor_reduce(
                    out=z1[:].rearrange("p (o2 k) -> p o2 k", o2=KS, k=K),
                    in_=twv, axis=AXIS.X, op=ALU.add)
                z1r = pool.tile([TP, KS * K], F32, tag="z1r")
                nc.scalar.activation(out=z1r[:], in_=z1[:], func=ACTF.Relu)

                # ---- h2 + bias
                tw2v = (tw[:, :KS * K * KS]
                        .rearrange("p (o3 k o2) -> p o3 k o2",
                                   o3=KS, k=K, o2=KS))
                z1_b = (z1r[:].rearrange("p (o2 k) -> p k o2", o2=KS, k=K)
                        .unsqueeze(1).to_broadcast([TP, KS, K, KS]))
                wh2_b = (wsb["wh2"][:TP, :]
                         .rearrange("p (o2 o3) -> p o3 o2", o2=KS, o3=KS)
                         .unsqueeze(2).to_broadcast([TP, KS, K, KS]))
                nc.vector.tensor_tensor(out=tw2v, in0=z1_b, in1=wh2_b,
                                        op=ALU.mult)
                z2 = pool.tile([TP, KS * K], F32, tag="z2")
                nc.vector.tensor_reduce(
                    out=z2[:].rearrange("p (o3 k) -> p o3 k", o3=KS, k=K),
                    in_=tw2v, axis=AXIS.X, op=ALU.add)
                bh2_b = (wsb["bh2"][:TP, :].unsqueeze(2)
                         .to_broadcast([TP, KS, K]))
                z2v = z2[:].rearrange("p (o3 k) -> p o3 k", o3=KS, k=K)
                nc.vector.tensor_tensor(out=z2v, in0=z2v, in1=bh2_b,
                                        op=ALU.add)

                # ---- softmax over o3
                mx = pool.tile([TP, K], F32, tag="mx")
                nc.vector.tensor_reduce(
                    out=mx[:],
                    in_=z2[:].rearrange("p (o3 k) -> p k o3", o3=KS, k=K),
                    axis=AXIS.X, op=ALU.max)
                mx_b = mx[:].unsqueeze(1).to_broadcast([TP, KS, K])
                ex = pool.tile([TP, KS * K], F32, tag="ex")
                exv = ex[:].rearrange("p (o3 k) -> p o3 k", o3=KS, k=K)
                nc.vector.tensor_tensor(out=exv, in0=z2v, in1=mx_b,
                                        op=ALU.subtract)
                exe = pool.tile([TP, KS * K], F32, tag="exe")
                nc.scalar.activation(out=exe[:], in_=ex[:], func=ACTF.Exp)
                exv = exe[:].rearrange("p (o3 k) -> p o3 k", o3=KS, k=K)
                sme = pool.tile([TP, K], F32, tag="sme")
                nc.vector.tensor_reduce(
                    out=sme[:],
                    in_=exe[:].rearrange("p (o3 k) -> p k o3", o3=KS, k=K),
                    axis=AXIS.X, op=ALU.add)
                rcs = pool.tile([TP, K], F32, tag="rcs")
                nc.vector.reciprocal(out=rcs[:], in_=sme[:])
                sc = pool.tile([TP, KS * K], F32, tag="sc")
                scv = sc[:].rearrange("p (ks k) -> p ks k", ks=KS, k=K)
                nc.vector.tensor_tensor(
                    out=scv, in0=exv,
                    in1=rcs[:].unsqueeze(1).to_broadcast([TP, KS, K]),
                    op=ALU.mult)

                # ---- weff[(c,h),k] = sum_ks sc[ks,k] * wbr[(c,h),ks]
                weff = pool.tile([TP, 96 * K], F32, tag="weff", bufs=2)
                KC = K // 2
                for kc in range(2):
                    tw3v = (tw[:, :96 * KC * KS]
                            .rearrange("p (ch k ks) -> p ch k ks",
                                       ch=96, k=KC, ks=KS))
                    sc_b = (sc[:].rearrange("p (ks k) -> p k ks", ks=KS, k=K)
                            [:, kc * KC:(kc + 1) * KC]
                            .unsqueeze(1).to_broadcast([TP, 96, KC, KS]))
                    wbr_b = (wsb["wbr"][:TP, :]
                             .rearrange("p (ch ks) -> p ch ks", ch=96, ks=KS)
                             .unsqueeze(2).to_broadcast([TP, 96, KC, KS]))
                    nc.vector.tensor_tensor(out=tw3v, in0=sc_b, in1=wbr_b,
                                            op=ALU.mult)
                    nc.vector.tensor_reduce(
                        out=weff[:].rearrange("p (ch k) -> p ch k",
                                              ch=96, k=K)
                        [:, :, kc * KC:(kc + 1) * KC],
                        in_=tw3v, axis=AXIS.X, op=ALU.add)

                # ---- pro[(h,d),k] = sum_c weff[(c,h),k] * local[c][d,k]
                pro = pool.tile([TP, HALF * 3 * K], F32, tag="pro", bufs=2)
                prov = pro[:].rearrange("p (h d k) -> p h d k",
                                        h=HALF, d=3, k=K)
                wev = weff[:].rearrange("p (c h k) -> p c h k",
                                        c=3, h=HALF, k=K)
                ptmp = pool.tile([TP, HALF * 3 * K], F32, tag="ptmp", bufs=1)
                ptv = ptmp[:].rearrange("p (h d k) -> p h d k",
                                        h=HALF, d=3, k=K)
                for c, src in enumerate((loc_b(pts[:], HALF), cen_b(HALF),
                                         loc_b(cross[:], HALF))):
                    we_b = wev[:, c].unsqueeze(2).to_broadcast(
                        [TP, HALF, 3, K])
                    if c == 0:
                        nc.vector.tensor_tensor(out=prov, in0=we_b, in1=src,
                                                op=ALU.mult)
                    else:
                        nc.vector.tensor_tensor(out=ptv, in0=we_b, in1=src,
                                                op=ALU.mult)
                        nc.vector.tensor_tensor(out=prov, in0=prov, in1=ptv,
                                                op=ALU.add)

                # ---- normalize over d, then mean over k -> feats [TP, (h,d)]
                q2 = pool.tile([TP, HALF * 3 * K], F32, tag="ptmp", bufs=1)
                nc.vector.tensor_tensor(out=q2[:], in0=pro[:], in1=pro[:],
                                        op=ALU.mult)
                ns = pool.tile([TP, HALF * K], F32, tag="ns")
                nc.vector.tensor_reduce(
                    out=ns[:].rearrange("p (h k) -> p h k", h=HALF, k=K),
                    in_=q2[:].rearrange("p (h d k) -> p h k d",
                                        h=HALF, d=3, k=K),
                    axis=AXIS.X, op=ALU.add)
                nr = pool.tile([TP, HALF * K], F32, tag="nr")
                nc.scalar.activation(out=nr[:], in_=ns[:], func=ACTF.Sqrt)
                nrc = pool.tile([TP, HALF * K], F32, tag="nrc")
                nc.vector.tensor_scalar(out=nrc[:], in0=nr[:], scalar1=1e-12,
                                        scalar2=None, op0=ALU.max)
                rcn = pool.tile([TP, HALF * K], F32, tag="rcn")
                nc.vector.reciprocal(out=rcn[:], in_=nrc[:])
                nmd = pool.tile([TP, HALF * 3 * K], F32, tag="nmd", bufs=1)
                rcn_b = (rcn[:].rearrange("p (h k) -> p h k", h=HALF, k=K)
                         .unsqueeze(2).to_broadcast([TP, HALF, 3, K]))
                nc.vector.tensor_tensor(
                    out=nmd[:].rearrange("p (h d k) -> p h d k",
                                         h=HALF, d=3, k=K),
                    in0=prov, in1=rcn_b, op=ALU.mult)
                fe = pool.tile([TP, HALF * 3], F32, tag="fe")
                nc.vector.tensor_reduce(
                    out=fe[:].rearrange("p (h d) -> p h d", h=HALF, d=3),
                    in_=nmd[:].rearrange("p (h d k) -> p h d k",
                                         h=HALF, d=3, k=K),
                    axis=AXIS.X, op=ALU.add)
                nc.vector.tensor_scalar(out=fe[:], in0=fe[:], scalar1=1.0 / K,
                                        scalar2=None, op0=ALU.mult)

                # ---- d1 = vn_lin(fe, wd_relu); f2 = vn_act(fe, d1)
                def emit_fc(src_tile, wtile, ci, co, dst_tag):
                    # dst[(o,d)] = sum_hi src[(hi,d)] * w[(o,hi)]
                    twv_ = (tw[:, :co * 3 * ci]
                            .rearrange("p (o d hi) -> p o d hi",
                                       o=co, d=3, hi=ci))
                    src_b = (src_tile[:].rearrange("p (h d) -> p d h",
                                                   h=ci, d=3)
                             .unsqueeze(1).to_broadcast([TP, co, 3, ci]))
                    w_b = (wtile[:TP, :co * ci]
                           .rearrange("p (o hi) -> p o hi", o=co, hi=ci)
                           .unsqueeze(2).to_broadcast([TP, co, 3, ci]))
                    nc.vector.tensor_tensor(out=twv_, in0=src_b, in1=w_b,
                                            op=ALU.mult)
                    dst = pool.tile([TP, co * 3], F32, tag=dst_tag)
                    nc.vector.tensor_reduce(
                        out=dst[:].rearrange("p (o d) -> p o d", o=co, d=3),
                        in_=twv_, axis=AXIS.X, op=ALU.add)
                    return dst

                d1 = emit_fc(fe, wsb["wdr"], HALF, HALF, "d1")
                f2 = _emit_vn_act(nc, pool, fe[:], d1[:], HALF, 1,
                                  out_scale_blend=True, tag="va2")

                # ---- out = vn_act(vn_lin(f2, w_un)*BN, vn_lin(f2, wd_un))
                u = emit_fc(f2, wsb["wun"], HALF, OUT, "u")
                du = emit_fc(f2, wsb["wdu"], HALF, OUT, "du")
                ot = _emit_vn_act(nc, pool, u[:], du[:], OUT, 1,
                                  out_scale_blend=True, tag="va3")

                nc.sync.dma_start(out=out[t * TP:(t + 1) * TP, :], in_=ot[:])

    if split_waits:
        _split_multi_waits(nc)
    return nc


def _prep_core_inputs(core, q_pts, s_pts, idx, weights):
    q = q_pts[core * NLOC:(core + 1) * NLOC].astype(np.float32)
    ix = idx[core * NLOC:(core + 1) * NLOC].astype(np.int32)
    qa = np.ascontiguousarray(
        q.reshape(NT, TP, 3).transpose(1, 0, 2).reshape(TP, NT * 3))
    ia = np.ascontiguousarray(
        ix.reshape(NT, TP, K).transpose(1, 0, 2).reshape(TP, NT * K))
    m = {"sp": np.ascontiguousarray(s_pts.astype(np.float32)),
         "qa": qa, "ia": ia}
    m.update(weights)
    return m


def kernel(q_pts, s_pts, s_feats, neighbor_indices, wb, w_vn, wd_vn,
           w_h1, w_h2, b_h2, wd_relu, w_un, wd_un):
    q_pts = np.asarray(q_pts, dtype=np.float32)
    s_pts = np.asarray(s_pts, dtype=np.float32)
    idx = np.asarray(neighbor_indices)
    wb = np.asarray(wb, np.float32); w_vn = np.asarray(w_vn, np.float32)
    wd_vn = np.asarray(wd_vn, np.float32); w_h1 = np.asarray(w_h1, np.float32)
    w_h2 = np.asarray(w_h2, np.float32); b_h2 = np.asarray(b_h2, np.float32)
    wd_relu = np.asarray(wd_relu, np.float32)
    w_un = np.asarray(w_un, np.float32); wd_un = np.asarray(wd_un, np.float32)

    def rep(a):
        return np.ascontiguousarray(np.asarray(a).reshape(1, -1),
                                    dtype=np.float32)

    # host-side weight packing (layouts documented at each use site)
    weights = {
        "wvn": rep((w_vn * BN)),                                  # (c,o)
        "wdv": rep(wd_vn),                                        # (c,o)
        "wh1": rep(w_h1 * BN * (1.0 - NEG)),                      # (o,o2)
        "wh2": rep(w_h2),                                         # (o2,o3)
        "bh2": rep(b_h2),                                         # (o3,)
        # wbr[(c,h),ks] = wb[c, ks*HALF+h]
        "wbr": rep(wb.reshape(3, KS, HALF).transpose(0, 2, 1)),   # (c,h,ks)
        # wdr[(ho,hi)] = wd_relu[hi,ho]
        "wdr": rep(wd_relu.T),
        "wun": rep((w_un * BN).T),                                # (o,hi)
        "wdu": rep(wd_un.T),                                      # (o,hi)
    }

    if "nc" not in _CACHE:
        _CACHE["nc"] = _build_bass()
    nc = _CACHE["nc"]

    in_maps = [_prep_core_inputs(c, q_pts, s_pts, idx, weights)
               for c in range(NCORES)]
    res = run_bass_kernel_spmd(nc, in_maps, core_ids=list(range(NCORES)))
    outs = [np.asarray(r["out"]).reshape(NLOC, OUT, 3) for r in res.results]
    return np.concatenate(outs, axis=0)


# revision 36
# speedup vs baseline: 1.2386x; 1.1003x over previous
"""Trainium2 Bass kernel for nn_AREConvFPN (vector-neuron GNN message passing).

Data-parallel over query points N=16000 across 8 NeuronCores (2000 pts/core).
Per-core layout: 16 tiles x 125 points-on-partitions; all per-point compute is
free-dim vector/scalar work using broadcast (step-0) access patterns; neighbor
gather of s_pts rows is done on-device with indirect DMA.

Self-contained: hardcodes all shapes; host side only slices/reshapes inputs.
"""

import math
import os
import numpy as np

DBG = os.environ.get("KDBG", "")

import concourse.bass as bass
import concourse.mybir as mybir
import concourse.tile as tile
from concourse.bass_utils import run_bass_kernel_spmd

F32 = mybir.dt.float32
I32 = mybir.dt.int32
ALU = mybir.AluOpType
ACTF = mybir.ActivationFunctionType
AXIS = mybir.AxisListType

NCORES = 8
N = 16000
N2 = 20000
K = 16
KS = 8          # kernel points
HALF = 32       # half of OUT
OUT = 64
NLOC = N // NCORES          # 2000 points per core
TP = 125                    # points per tile (partitions used)
NT = NLOC // TP             # 16 tiles
EPS = 1e-6
NEG = 0.2
BN = np.float32(1.0 / math.sqrt(1.0 + 1e-5))

_CACHE = {}


def _emit_vn_act(nc, pool, p_ap, d_ap, ch, nd, out_scale_blend, tag):
    """VN leaky relu on tensors laid out [TP, (ch, d=3) (+ maybe k)] with d at
    stride nd_inner. p_ap/d_ap are APs over [TP, ch*3*(extra)] tiles whose free
    layout is (ch, 3, inner) with inner size nd (nd=1 for no-k case).
    Returns a new tile AP [TP, ch*3*nd] holding the result.
    out_scale_blend: if True apply 0.2/0.8 blend; else caller folded 0.8 and we
    apply 0.25*p + sel (w/ 0.8 folded downstream).
    """
    FREE = ch * 3 * nd
    CF = ch * nd

    def v3(ap):  # [TP, ch, 3, nd]
        return ap.rearrange("p (c d i) -> p c d i", c=ch, d=3, i=nd)

    def v_red(ap):  # [TP, ch, nd, 3]  (d innermost for reduce)
        return ap.rearrange("p (c d i) -> p c i d", c=ch, d=3, i=nd)

    pd = pool.tile([TP, FREE], F32, tag=f"{tag}_pd", bufs=2)
    nc.vector.tensor_tensor(out=pd[:], in0=p_ap, in1=d_ap, op=ALU.mult)
    dot = pool.tile([TP, CF], F32, tag=f"{tag}_dot")
    nc.vector.tensor_reduce(
        out=dot[:].rearrange("p (c i) -> p c i", c=ch, i=nd),
        in_=v_red(pd[:]), axis=AXIS.X, op=ALU.add)
    dd = pool.tile([TP, FREE], F32, tag=f"{tag}_pd", bufs=2)
    nc.vector.tensor_tensor(out=dd[:], in0=d_ap, in1=d_ap, op=ALU.mult)
    dsq = pool.tile([TP, CF], F32, tag=f"{tag}_dsq")
    nc.vector.tensor_reduce(
        out=dsq[:].rearrange("p (c i) -> p c i", c=ch, i=nd),
        in_=v_red(dd[:]), axis=AXIS.X, op=ALU.add)
    den = pool.tile([TP, CF], F32, tag=f"{tag}_den")
    nc.vector.tensor_scalar(out=den[:], in0=dsq[:], scalar1=EPS, scalar2=None,
                            op0=ALU.add)
    rcp = pool.tile([TP, CF], F32, tag=f"{tag}_rcp")
    nc.vector.reciprocal(out=rcp[:], in_=den[:])
    rat = pool.tile([TP, CF], F32, tag=f"{tag}_rat")
    nc.vector.tensor_tensor(out=rat[:], in0=dot[:], in1=rcp[:], op=ALU.mult)

    # corr = p - rat*d
    tmp = pool.tile([TP, FREE], F32, tag=f"{tag}_pd", bufs=2)
    rat_b = (rat[:].rearrange("p (c i) -> p c i", c=ch, i=nd)
             .unsqueeze(2).to_broadcast([TP, ch, 3, nd]))
    nc.vector.tensor_tensor(out=v3(tmp[:]), in0=rat_b, in1=v3(d_ap), op=ALU.mult)
    corr = pool.tile([TP, FREE], F32, tag=f"{tag}_corr", bufs=2)
    nc.vector.tensor_tensor(out=corr[:], in0=p_ap, in1=tmp[:], op=ALU.subtract)

    mask = pool.tile([TP, FREE], mybir.dt.uint8, tag=f"{tag}_mask")
    dot_b = (dot[:].rearrange("p (c i) -> p c i", c=ch, i=nd)
             .unsqueeze(2).to_broadcast([TP, ch, 3, nd]))
    nc.vector.tensor_scalar(out=v3(mask[:]), in0=dot_b, scalar1=0.0,
                            scalar2=None, op0=ALU.is_ge)
    sel = pool.tile([TP, FREE], F32, tag=f"{tag}_sel", bufs=2)
    nc.vector.tensor_copy(out=sel[:], in_=corr[:])
    nc.vector.copy_predicated(out=sel[:], mask=mask[:], data=p_ap)

    res = pool.tile([TP, FREE], F32, tag=f"{tag}_res", bufs=2)
    if out_scale_blend:
        # res = 0.2*p + 0.8*sel
        t1 = pool.tile([TP, FREE], F32, tag=f"{tag}_pd", bufs=2)
        nc.scalar.activation(out=t1[:], in_=p_ap, func=ACTF.Copy, scale=float(NEG))
        t2 = pool.tile([TP, FREE], F32, tag=f"{tag}_corr", bufs=2)
        nc.scalar.activation(out=t2[:], in_=sel[:], func=ACTF.Copy,
                             scale=float(1.0 - NEG))
        nc.vector.tensor_tensor(out=res[:], in0=t1[:], in1=t2[:], op=ALU.add)
    else:
        # res = 0.25*p + sel   (0.8 folded into the next weights)
        t1 = pool.tile([TP, FREE], F32, tag=f"{tag}_pd", bufs=2)
        nc.scalar.activation(out=t1[:], in_=p_ap, func=ACTF.Copy,
                             scale=float(NEG / (1.0 - NEG)))
        nc.vector.tensor_tensor(out=res[:], in0=t1[:], in1=sel[:], op=ALU.add)
    return res


def _split_multi_waits(nc):
    """TRN2 compute/DMA instructions encode at most one semaphore wait.
    Tile sometimes emits several; hoist the extras onto standalone
    EventSemaphore instructions placed just before (same engine stream)."""
    n = 0
    for f in nc.m.functions:
        for blk in f.blocks:
            out = []
            changed = False
            for inst in blk.instructions:
                si = inst.sync_info
                if si is not None and si.on_wait and len(si.on_wait) > 1:
                    waits = list(si.on_wait)
                    for w in waits[:-1]:
                        ev = mybir.InstEventSemaphore(
                            name=f"sw-{n}-{inst.name}", engine=inst.engine,
                            ins=[], outs=[],
                            sync_info=mybir.SyncInfo(on_wait=[w],
                                                     on_update=[]))
                        out.append(ev)
                        n += 1
                    inst.sync_info = mybir.SyncInfo(
                        on_wait=[waits[-1]], on_update=list(si.on_update))
                    changed = True
                out.append(inst)
            if changed:
                blk.instructions = out
    return n


def _build_bass(split_waits=True):
    nc = bass.Bass()

    sp = nc.declare_dram_parameter("sp", [N2, 4], F32, isOutput=False)
    qa = nc.declare_dram_parameter("qa", [TP, NT * 3], F32, isOutput=False)
    ia = nc.declare_dram_parameter("ia", [TP, NT * K], I32, isOutput=False)
    w_shapes = dict(wvn=48, wdv=48, wh1=128, wh2=64, bh2=8, wbr=768,
                    wdr=1024, wun=2048, wdu=2048)
    wparams = {k: nc.declare_dram_parameter(k, [1, v], F32, isOutput=False)
               for k, v in w_shapes.items()}
    out = nc.declare_dram_parameter("out", [NLOC, OUT * 3], F32, isOutput=True)

    with tile.TileContext(nc) as tc:
        with (
            tc.tile_pool(name="wts", bufs=1) as wpool,
            tc.tile_pool(name="g", bufs=NT) as gpool,
            tc.tile_pool(name="work", bufs=2) as pool,
            tc.tile_pool(name="jumbo", bufs=1) as jpool,
        ):
            wsb = {}
            for k_, v in w_shapes.items():
                wsb[k_] = wpool.tile([128, v], F32, tag=f"w_{k_}",
                                     name=f"w_{k_}")
                nc.sync.dma_start(
                    out=wsb[k_][:],
                    in_=wparams[k_][:, :].to_broadcast([128, v]))
            qsb = wpool.tile([TP, NT * 3], F32, tag="qsb")
            nc.sync.dma_start(out=qsb[:], in_=qa[:, :])
            isb = wpool.tile([TP, NT * K], I32, tag="isb")
            nc.sync.dma_start(out=isb[:], in_=ia[:, :])

            # "touch" every staged tile once per consumer engine so later
            # instructions never need more than one DMA-sem wait each
            # (TRN2 allows a single sync-wait per compute instruction).
            dummy = wpool.tile([128, 16], F32, tag="dummy")
            dummyi = wpool.tile([128, 2], I32, tag="dummyi")
            for i, k_ in enumerate(w_shapes):
                nc.vector.tensor_scalar(out=dummy[:, i:i + 1],
                                        in0=wsb[k_][:, :1], scalar1=1.0,
                                        scalar2=None, op0=ALU.mult)
            nc.vector.tensor_scalar(out=dummy[:TP, 10:11], in0=qsb[:, :1],
                                    scalar1=1.0, scalar2=None, op0=ALU.mult)
            nc.gpsimd.tensor_scalar(out=dummyi[:TP, :1], in0=isb[:, :1],
                                    scalar1=1, scalar2=None, op0=ALU.mult)

            for t in range(NT):
                # ---- gather neighbors: G[p, k*3+d] = s_pts[idx[p,k], d]
                # HW indirect DMA uses one index per partition, so issue one
                # DMA per neighbor slot k.
                G = gpool.tile([TP, K * 4], F32, tag="G")
                for k in range(K):
                    nc.gpsimd.indirect_dma_start(
                        out=G[:, k * 4:(k + 1) * 4], out_offset=None,
                        in_=sp[:, :],
                        in_offset=bass.IndirectOffsetOnAxis(
                            ap=isb[:, t * K + k:t * K + k + 1], axis=0))

                # ---- pts = G - q  (broadcast q over k)
                pts = pool.tile([TP, K * 3], F32, tag="pts")
                q_b = (qsb[:, t * 3:(t + 1) * 3].unsqueeze(1)
                       .to_broadcast([TP, K, 3]))
                nc.vector.tensor_tensor(
                    out=pts[:].rearrange("p (k d) -> p k d", k=K, d=3),
                    in0=G[:].rearrange("p (k e) -> p k e", k=K, e=4)[:, :, :3],
                    in1=q_b, op=ALU.subtract)

                def pts_kd():
                    return pts[:].rearrange("p (k d) -> p k d", k=K, d=3)

                def pts_dk():
                    return pts[:].rearrange("p (k d) -> p d k", k=K, d=3)

                # ---- centers = mean_k pts  [TP, 3]
                cen = pool.tile([TP, 3], F32, tag="cen")
                nc.vector.tensor_reduce(out=cen[:], in_=pts_dk(),
                                        axis=AXIS.X, op=ALU.add)
                nc.vector.tensor_scalar(out=cen[:], in0=cen[:],
                                        scalar1=1.0 / K, scalar2=None,
                                        op0=ALU.mult)

                # ---- cross = pts x cen  [TP, (k,d)]
                cross = pool.tile([TP, K * 3], F32, tag="cross")
                crv = cross[:].rearrange("p (k d) -> p k d", k=K, d=3)
                ct1 = pool.tile([TP, K], F32, tag="ct1")
                ct2 = pool.tile([TP, K], F32, tag="ct2")
                for dd in range(3):
                    d1, d2 = (dd + 1) % 3, (dd + 2) % 3
                    nc.vector.tensor_tensor(
                        out=ct1[:], in0=pts_kd()[:, :, d1],
                        in1=cen[:, d2:d2 + 1].to_broadcast([TP, K]),
                        op=ALU.mult)
                    nc.vector.tensor_tensor(
                        out=ct2[:], in0=pts_kd()[:, :, d2],
                        in1=cen[:, d1:d1 + 1].to_broadcast([TP, K]),
                        op=ALU.mult)
                    nc.vector.tensor_tensor(
                        out=crv[:, :, dd], in0=ct1[:], in1=ct2[:],
                        op=ALU.subtract)

                if DBG == "local":
                    nc.sync.dma_start(out=out[t * TP:(t + 1) * TP, :48],
                                      in_=pts[:])
                    nc.sync.dma_start(out=out[t * TP:(t + 1) * TP, 48:96],
                                      in_=cross[:])
                    nc.sync.dma_start(out=out[t * TP:(t + 1) * TP, 96:99],
                                      in_=cen[:])
                    nc.sync.dma_start(
                        out=out[t * TP:(t + 1) * TP, 99:147]
                        .rearrange("p (k d) -> p k d", k=K, d=3),
                        in_=G[:].rearrange("p (k e) -> p k e", k=K, e=4)
                        [:, :, :3])
                    continue

                def cross_dk():
                    return cross[:].rearrange("p (k d) -> p d k", k=K, d=3)

                def cen_b(o):  # [TP, o, 3, K] broadcast
                    return (cen[:].unsqueeze(1).unsqueeze(3)
                            .to_broadcast([TP, o, 3, K]))

                def loc_b(ap, o):  # local (k,d) tile -> [TP, o, 3, K]
                    return (ap.rearrange("p (k d) -> p d k", k=K, d=3)
                            .unsqueeze(1).to_broadcast([TP, o, 3, K]))

                # ---- p = vn_lin(local, wvn) (BN folded), layout (o, d, k)
                def emit_vnlin(wtile, o, dst_tag):
                    dst = pool.tile([TP, o * 3 * K], F32, tag=dst_tag)
                    dv = dst[:].rearrange("p (o d k) -> p o d k", o=o, d=3, k=K)
                    wv = wtile[:TP, :].rearrange("p (c o) -> p c o", c=3, o=o)
                    tmp = pool.tile([TP, o * 3 * K], F32, tag="vl_tmp", bufs=2)
                    tv = tmp[:].rearrange("p (o d k) -> p o d k", o=o, d=3, k=K)
                    for c, src in enumerate((loc_b(pts[:], o), cen_b(o),
                                             loc_b(cross[:], o))):
                        wb_ = (wv[:, c].unsqueeze(2).unsqueeze(3)
                               .to_broadcast([TP, o, 3, K]))
                        if c == 0:
                            nc.vector.tensor_tensor(out=dv, in0=src, in1=wb_,
                                                    op=ALU.mult)
                        else:
                            nc.vector.tensor_tensor(out=tv, in0=src, in1=wb_,
                                                    op=ALU.mult)
                            nc.vector.tensor_tensor(out=dv, in0=dv, in1=tv,
                                                    op=ALU.add)
                    return dst

                P_ = emit_vnlin(wsb["wvn"], 2 * KS, "P_")
                D_ = emit_vnlin(wsb["wdv"], 2 * KS, "D_")

                # ---- s = vn_act(p, d) with 0.8 folded into wh1
                S_ = _emit_vn_act(nc, pool, P_[:], D_[:], 2 * KS, K,
                                  out_scale_blend=False, tag="va1")

                # ---- snorm = ||s||_d  [TP, (o,k)]
                ss = pool.tile([TP, 2 * KS * 3 * K], F32, tag="va1_pd",
                               bufs=2)
                nc.vector.tensor_tensor(out=ss[:], in0=S_[:], in1=S_[:],
                                        op=ALU.mult)
                nsq = pool.tile([TP, 2 * KS * K], F32, tag="nsq")
                nc.vector.tensor_reduce(
                    out=nsq[:].rearrange("p (o k) -> p o k", o=2 * KS, k=K),
                    in_=ss[:].rearrange("p (o d k) -> p o k d",
                                        o=2 * KS, d=3, k=K),
                    axis=AXIS.X, op=ALU.add)
                sn = pool.tile([TP, 2 * KS * K], F32, tag="sn")
                nc.scalar.activation(out=sn[:], in_=nsq[:], func=ACTF.Sqrt)

                # ---- h1: z1[o2,k] = relu(sum_o sn[o,k]*wh1[o,o2])
                tw = jpool.tile([TP, 3072], F32, tag="jumbo")
                twv = (tw[:, :KS * K * 2 * KS]
                       .rearrange("p (o2 k o) -> p o2 k o",
                                  o2=KS, k=K, o=2 * KS))
                sn_b = (sn[:].rearrange("p (o k) -> p k o", o=2 * KS, k=K)
                        .unsqueeze(1).to_broadcast([TP, KS, K, 2 * KS]))
                wh1_b = (wsb["wh1"][:TP, :]
                         .rearrange("p (o o2) -> p o2 o", o=2 * KS, o2=KS)
                         .unsqueeze(2).to_broadcast([TP, KS, K, 2 * KS]))
                nc.vector.tensor_tensor(out=twv, in0=sn_b, in1=wh1_b,
                                        op=ALU.mult)
                z1 = pool.tile([TP, KS * K], F32, tag="z1")
                nc.vector.tensor_reduce(
                    out=z1[:].rearrange("p (o2 k) -> p o2 k", o2=KS, k=K),
                    in_=twv, axis=AXIS.X, op=ALU.add)
                z1r = pool.tile([TP, KS * K], F32, tag="z1r")
                nc.scalar.activation(out=z1r[:], in_=z1[:], func=ACTF.Relu)

                # ---- h2 + bias
                tw2v = (tw[:, :KS * K * KS]
                        .rearrange("p (o3 k o2) -> p o3 k o2",
                                   o3=KS, k=K, o2=KS))
                z1_b = (z1r[:].rearrange("p (o2 k) -> p k o2", o2=KS, k=K)
                        .unsqueeze(1).to_broadcast([TP, KS, K, KS]))
                wh2_b = (wsb["wh2"][:TP, :]
                         .rearrange("p (o2 o3) -> p o3 o2", o2=KS, o3=KS)
                         .unsqueeze(2).to_broadcast([TP, KS, K, KS]))
                nc.vector.tensor_tensor(out=tw2v, in0=z1_b, in1=wh2_b,
                                        op=ALU.mult)
                z2 = pool.tile([TP, KS * K], F32, tag="z2")
                nc.vector.tensor_reduce(
                    out=z2[:].rearrange("p (o3 k) -> p o3 k", o3=KS, k=K),
                    in_=tw2v, axis=AXIS.X, op=ALU.add)
                bh2_b = (wsb["bh2"][:TP, :].unsqueeze(2)
                         .to_broadcast([TP, KS, K]))
                z2v = z2[:].rearrange("p (o3 k) -> p o3 k", o3=KS, k=K)
                nc.vector.tensor_tensor(out=z2v, in0=z2v, in1=bh2_b,
                                        op=ALU.add)

                # ---- softmax over o3
                mx = pool.tile([TP, K], F32, tag="mx")
                nc.vector.tensor_reduce(
                    out=mx[:],
                    in_=z2[:].rearrange("p (o3 k) -> p k o3", o3=KS, k=K),
                    axis=AXIS.X, op=ALU.max)
                mx_b = mx[:].unsqueeze(1).to_broadcast([TP, KS, K])
                ex = pool.tile([TP, KS * K], F32, tag="ex")
                exv = ex[:].rearrange("p (o3 k) -> p o3 k", o3=KS, k=K)
                nc.vector.tensor_tensor(out=exv, in0=z2v, in1=mx_b,
                                        op=ALU.subtract)
                exe = pool.tile([TP, KS * K], F32, tag="exe")
                nc.scalar.activation(out=exe[:], in_=ex[:], func=ACTF.Exp)
                exv = exe[:].rearrange("p (o3 k) -> p o3 k", o3=KS, k=K)
                sme = pool.tile([TP, K], F32, tag="sme")
                nc.vector.tensor_reduce(
                    out=sme[:],
                    in_=exe[:].rearrange("p (o3 k) -> p k o3", o3=KS, k=K),
                    axis=AXIS.X, op=ALU.add)
                rcs = pool.tile([TP, K], F32, tag="rcs")
                nc.vector.reciprocal(out=rcs[:], in_=sme[:])
                sc = pool.tile([TP, KS * K], F32, tag="sc")
                scv = sc[:].rearrange("p (ks k) -> p ks k", ks=KS, k=K)
                nc.vector.tensor_tensor(
                    out=scv, in0=exv,
                    in1=rcs[:].unsqueeze(1).to_broadcast([TP, KS, K]),
                    op=ALU.mult)

                # ---- weff[(c,h),k] = sum_ks sc[ks,k] * wbr[(c,h),ks]
                weff = pool.tile([TP, 96 * K], F32, tag="weff", bufs=2)
                KC = K // 4
                for kc in range(4):
                    tw3v = (tw[:, :96 * KC * KS]
                            .rearrange("p (ch k ks) -> p ch k ks",
                                       ch=96, k=KC, ks=KS))
                    sc_b = (sc[:].rearrange("p (ks k) -> p k ks", ks=KS, k=K)
                            [:, kc * KC:(kc + 1) * KC]
                            .unsqueeze(1).to_broadcast([TP, 96, KC, KS]))
                    wbr_b = (wsb["wbr"][:TP, :]
                             .rearrange("p (ch ks) -> p ch ks", ch=96, ks=KS)
                             .unsqueeze(2).to_broadcast([TP, 96, KC, KS]))
                    nc.vector.tensor_tensor(out=tw3v, in0=sc_b, in1=wbr_b,
                                            op=ALU.mult)
                    nc.vector.tensor_reduce(
                        out=weff[:].rearrange("p (ch k) -> p ch k",
                                              ch=96, k=K)
                        [:, :, kc * KC:(kc + 1) * KC],
                        in_=tw3v, axis=AXIS.X, op=ALU.add)

                # ---- pro[(h,d),k] = sum_c weff[(c,h),k] * local[c][d,k]
                pro = pool.tile([TP, HALF * 3 * K], F32, tag="pro", bufs=2)
                prov = pro[:].rearrange("p (h d k) -> p h d k",
                                        h=HALF, d=3, k=K)
                wev = weff[:].rearrange("p (c h k) -> p c h k",
                                        c=3, h=HALF, k=K)
                ptmp = pool.tile([TP, HALF * 3 * K], F32, tag="ptmp", bufs=2)
                ptv = ptmp[:].rearrange("p (h d k) -> p h d k",
                                        h=HALF, d=3, k=K)
                for c, src in enumerate((loc_b(pts[:], HALF), cen_b(HALF),
                                         loc_b(cross[:], HALF))):
                    we_b = wev[:, c].unsqueeze(2).to_broadcast(
                        [TP, HALF, 3, K])
                    if c == 0:
                        nc.vector.tensor_tensor(out=prov, in0=we_b, in1=src,
                                                op=ALU.mult)
                    else:
                        nc.vector.tensor_tensor(out=ptv, in0=we_b, in1=src,
                                                op=ALU.mult)
                        nc.vector.tensor_tensor(out=prov, in0=prov, in1=ptv,
                                                op=ALU.add)

                # ---- normalize over d, then mean over k -> feats [TP, (h,d)]
                q2 = pool.tile([TP, HALF * 3 * K], F32, tag="ptmp", bufs=2)
                nc.vector.tensor_tensor(out=q2[:], in0=pro[:], in1=pro[:],
                                        op=ALU.mult)
                ns = pool.tile([TP, HALF * K], F32, tag="ns")
                nc.vector.tensor_reduce(
                    out=ns[:].rearrange("p (h k) -> p h k", h=HALF, k=K),
                    in_=q2[:].rearrange("p (h d k) -> p h k d",
                                        h=HALF, d=3, k=K),
                    axis=AXIS.X, op=ALU.add)
                nr = pool.tile([TP, HALF * K], F32, tag="nr")
                nc.scalar.activation(out=nr[:], in_=ns[:], func=ACTF.Sqrt)
                nrc = pool.tile([TP, HALF * K], F32, tag="nrc")
                nc.vector.tensor_scalar(out=nrc[:], in0=nr[:], scalar1=1e-12,
                                        scalar2=None, op0=ALU.max)
                rcn = pool.tile([TP, HALF * K], F32, tag="rcn")
                nc.vector.reciprocal(out=rcn[:], in_=nrc[:])
                nmd = pool.tile([TP, HALF * 3 * K], F32, tag="nmd", bufs=2)
                rcn_b = (rcn[:].rearrange("p (h k) -> p h k", h=HALF, k=K)
                         .unsqueeze(2).to_broadcast([TP, HALF, 3, K]))
                nc.vector.tensor_tensor(
                    out=nmd[:].rearrange("p (h d k) -> p h d k",
                                         h=HALF, d=3, k=K),
                    in0=prov, in1=rcn_b, op=ALU.mult)
                fe = pool.tile([TP, HALF * 3], F32, tag="fe")
                nc.vector.tensor_reduce(
                    out=fe[:].rearrange("p (h d) -> p h d", h=HALF, d=3),
                    in_=nmd[:].rearrange("p (h d k) -> p h d k",
                                         h=HALF, d=3, k=K),
                    axis=AXIS.X, op=ALU.add)
                nc.vector.tensor_scalar(out=fe[:], in0=fe[:], scalar1=1.0 / K,
                                        scalar2=None, op0=ALU.mult)

                # ---- d1 = vn_lin(fe, wd_relu); f2 = vn_act(fe, d1)
                def emit_fc(src_tile, wtile, ci, co, dst_tag):
                    # dst[(o,d)] = sum_hi src[(hi,d)] * w[(o,hi)]
                    dst = pool.tile([TP, co * 3], F32, tag=dst_tag)
                    OC = min(co, 3072 // (3 * ci))
                    src_b = (src_tile[:].rearrange("p (h d) -> p d h",
                                                   h=ci, d=3)
                             .unsqueeze(1).to_broadcast([TP, OC, 3, ci]))
                    for oc in range(0, co, OC):
                        twv_ = (tw[:, :OC * 3 * ci]
                                .rearrange("p (o d hi) -> p o d hi",
                                           o=OC, d=3, hi=ci))
                        w_b = (wtile[:TP, oc * ci:(oc + OC) * ci]
                               .rearrange("p (o hi) -> p o hi", o=OC, hi=ci)
                               .unsqueeze(2).to_broadcast([TP, OC, 3, ci]))
                        nc.vector.tensor_tensor(out=twv_, in0=src_b, in1=w_b,
                                                op=ALU.mult)
                        nc.vector.tensor_reduce(
                            out=dst[:].rearrange("p (o d) -> p o d",
                                                 o=co, d=3)
                            [:, oc:oc + OC],
                            in_=twv_, axis=AXIS.X, op=ALU.add)
                    return dst

                d1 = emit_fc(fe, wsb["wdr"], HALF, HALF, "d1")
                f2 = _emit_vn_act(nc, pool, fe[:], d1[:], HALF, 1,
                                  out_scale_blend=True, tag="va2")

                # ---- out = vn_act(vn_lin(f2, w_un)*BN, vn_lin(f2, wd_un))
                u = emit_fc(f2, wsb["wun"], HALF, OUT, "u")
                du = emit_fc(f2, wsb["wdu"], HALF, OUT, "du")
                ot = _emit_vn_act(nc, pool, u[:], du[:], OUT, 1,
                                  out_scale_blend=True, tag="va3")

                nc.sync.dma_start(out=out[t * TP:(t + 1) * TP, :], in_=ot[:])

    if split_waits:
        _split_multi_waits(nc)
    return nc


def _prep_core_inputs(core, q_pts, s_pts, idx, weights):
    q = q_pts[core * NLOC:(core + 1) * NLOC].astype(np.float32)
    ix = idx[core * NLOC:(core + 1) * NLOC].astype(np.int32)
    qa = np.ascontiguousarray(
        q.reshape(NT, TP, 3).transpose(1, 0, 2).reshape(TP, NT * 3))
    ia = np.ascontiguousarray(
        ix.reshape(NT, TP, K).transpose(1, 0, 2).reshape(TP, NT * K))
    sp4 = np.zeros((s_pts.shape[0], 4), np.float32)
    sp4[:, :3] = s_pts
    m = {"sp": sp4, "qa": qa, "ia": ia}
    m.update(weights)
    return m


def kernel(q_pts, s_pts, s_feats, neighbor_indices, wb, w_vn, wd_vn,
           w_h1, w_h2, b_h2, wd_relu, w_un, wd_un):
    q_pts = np.asarray(q_pts, dtype=np.float32)
    s_pts = np.asarray(s_pts, dtype=np.float32)
    idx = np.asarray(neighbor_indices)
    wb = np.asarray(wb, np.float32); w_vn = np.asarray(w_vn, np.float32)
    wd_vn = np.asarray(wd_vn, np.float32); w_h1 = np.asarray(w_h1, np.float32)
    w_h2 = np.asarray(w_h2, np.float32); b_h2 = np.asarray(b_h2, np.float32)
    wd_relu = np.asarray(wd_relu, np.float32)
    w_un = np.asarray(w_un, np.float32); wd_un = np.asarray(wd_un, np.float32)

    def rep(a):
        return np.ascontiguousarray(np.asarray(a).reshape(1, -1),
                                    dtype=np.float32)

    # host-side weight packing (layouts documented at each use site)
    weights = {
        "wvn": rep((w_vn * BN)),                                  # (c,o)
        "wdv": rep(wd_vn),                                        # (c,o)
        "wh1": rep(w_h1 * BN * (1.0 - NEG)),                      # (o,o2)
        "wh2": rep(w_h2),                                         # (o2,o3)
        "bh2": rep(b_h2),                                         # (o3,)
        # wbr[(c,h),ks] = wb[c, ks*HALF+h]
        "wbr": rep(wb.reshape(3, KS, HALF).transpose(0, 2, 1)),   # (c,h,ks)
        # wdr[(ho,hi)] = wd_relu[hi,ho]
        "wdr": rep(wd_relu.T),
        "wun": rep((w_un * BN).T),                                # (o,hi)
        "wdu": rep(wd_un.T),                                      # (o,hi)
    }

    if "nc" not in _CACHE:
        _CACHE["nc"] = _build_bass()
    nc = _CACHE["nc"]

    in_maps = [_prep_core_inputs(c, q_pts, s_pts, idx, weights)
               for c in range(NCORES)]
    res = run_bass_kernel_spmd(nc, in_maps, core_ids=list(range(NCORES)))
    outs = [np.asarray(r["out"]).reshape(NLOC, OUT, 3) for r in res.results]
    return np.concatenate(outs, axis=0)


# revision 37
# speedup vs baseline: 1.5225x; 1.2292x over previous
"""Trainium2 Bass kernel for nn_AREConvFPN (vector-neuron GNN message passing).

Data-parallel over query points N=16000 across 8 NeuronCores (2000 pts/core).
Per-core layout: 16 tiles x 125 points-on-partitions; all per-point compute is
free-dim vector/scalar work using broadcast (step-0) access patterns; neighbor
gather of s_pts rows is done on-device with indirect DMA.

Self-contained: hardcodes all shapes; host side only slices/reshapes inputs.
"""

import math
import os
import numpy as np

DBG = os.environ.get("KDBG", "")

import concourse.bass as bass
import concourse.mybir as mybir
import concourse.tile as tile
from concourse.bass_utils import run_bass_kernel_spmd

F32 = mybir.dt.float32
I32 = mybir.dt.int32
ALU = mybir.AluOpType
ACTF = mybir.ActivationFunctionType
AXIS = mybir.AxisListType

NCORES = 8
N = 16000
N2 = 20000
K = 16
KS = 8          # kernel points
HALF = 32       # half of OUT
OUT = 64
NLOC = N // NCORES          # 2000 points per core
TP = 125                    # points per tile (partitions used)
NT = NLOC // TP             # 16 tiles
EPS = 1e-6
NEG = 0.2
BN = np.float32(1.0 / math.sqrt(1.0 + 1e-5))

_CACHE = {}


def _emit_vn_act(nc, pool, p_ap, d_ap, ch, nd, out_scale_blend, tag):
    """VN leaky relu on tensors laid out [TP, (ch, d=3) (+ maybe k)] with d at
    stride nd_inner. p_ap/d_ap are APs over [TP, ch*3*(extra)] tiles whose free
    layout is (ch, 3, inner) with inner size nd (nd=1 for no-k case).
    Returns a new tile AP [TP, ch*3*nd] holding the result.
    out_scale_blend: if True apply 0.2/0.8 blend; else caller folded 0.8 and we
    apply 0.25*p + sel (w/ 0.8 folded downstream).
    """
    FREE = ch * 3 * nd
    CF = ch * nd

    def v3(ap):  # [TP, ch, 3, nd]
        return ap.rearrange("p (c d i) -> p c d i", c=ch, d=3, i=nd)

    def v_red(ap):  # [TP, ch, nd, 3]  (d innermost for reduce)
        return ap.rearrange("p (c d i) -> p c i d", c=ch, d=3, i=nd)

    pd = pool.tile([TP, FREE], F32, tag=f"{tag}_pd", bufs=2)
    nc.vector.tensor_tensor(out=pd[:], in0=p_ap, in1=d_ap, op=ALU.mult)
    dot = pool.tile([TP, CF], F32, tag=f"{tag}_dot")
    nc.vector.tensor_reduce(
        out=dot[:].rearrange("p (c i) -> p c i", c=ch, i=nd),
        in_=v_red(pd[:]), axis=AXIS.X, op=ALU.add)
    dd = pool.tile([TP, FREE], F32, tag=f"{tag}_pd", bufs=2)
    nc.vector.tensor_tensor(out=dd[:], in0=d_ap, in1=d_ap, op=ALU.mult)
    dsq = pool.tile([TP, CF], F32, tag=f"{tag}_dsq")
    nc.vector.tensor_reduce(
        out=dsq[:].rearrange("p (c i) -> p c i", c=ch, i=nd),
        in_=v_red(dd[:]), axis=AXIS.X, op=ALU.add)
    den = pool.tile([TP, CF], F32, tag=f"{tag}_den")
    nc.vector.tensor_scalar(out=den[:], in0=dsq[:], scalar1=EPS, scalar2=None,
                            op0=ALU.add)
    rcp = pool.tile([TP, CF], F32, tag=f"{tag}_rcp")
    nc.vector.reciprocal(out=rcp[:], in_=den[:])
    rat = pool.tile([TP, CF], F32, tag=f"{tag}_rat")
    nc.vector.tensor_tensor(out=rat[:], in0=dot[:], in1=rcp[:], op=ALU.mult)

    # corr = p - rat*d
    tmp = pool.tile([TP, FREE], F32, tag=f"{tag}_pd", bufs=2)
    rat_b = (rat[:].rearrange("p (c i) -> p c i", c=ch, i=nd)
             .unsqueeze(2).to_broadcast([TP, ch, 3, nd]))
    nc.vector.tensor_tensor(out=v3(tmp[:]), in0=rat_b, in1=v3(d_ap), op=ALU.mult)
    corr = pool.tile([TP, FREE], F32, tag=f"{tag}_corr", bufs=2)
    nc.vector.tensor_tensor(out=corr[:], in0=p_ap, in1=tmp[:], op=ALU.subtract)

    mask = pool.tile([TP, FREE], mybir.dt.uint8, tag=f"{tag}_mask")
    dot_b = (dot[:].rearrange("p (c i) -> p c i", c=ch, i=nd)
             .unsqueeze(2).to_broadcast([TP, ch, 3, nd]))
    nc.vector.tensor_scalar(out=v3(mask[:]), in0=dot_b, scalar1=0.0,
                            scalar2=None, op0=ALU.is_ge)
    sel = pool.tile([TP, FREE], F32, tag=f"{tag}_sel", bufs=2)
    nc.vector.tensor_copy(out=sel[:], in_=corr[:])
    nc.vector.copy_predicated(out=sel[:], mask=mask[:], data=p_ap)

    res = pool.tile([TP, FREE], F32, tag=f"{tag}_res", bufs=2)
    if out_scale_blend:
        # res = 0.2*p + 0.8*sel
        t1 = pool.tile([TP, FREE], F32, tag=f"{tag}_pd", bufs=2)
        nc.scalar.activation(out=t1[:], in_=p_ap, func=ACTF.Copy, scale=float(NEG))
        t2 = pool.tile([TP, FREE], F32, tag=f"{tag}_corr", bufs=2)
        nc.scalar.activation(out=t2[:], in_=sel[:], func=ACTF.Copy,
                             scale=float(1.0 - NEG))
        nc.vector.tensor_tensor(out=res[:], in0=t1[:], in1=t2[:], op=ALU.add)
    else:
        # res = 0.25*p + sel   (0.8 folded into the next weights)
        t1 = pool.tile([TP, FREE], F32, tag=f"{tag}_pd", bufs=2)
        nc.scalar.activation(out=t1[:], in_=p_ap, func=ACTF.Copy,
                             scale=float(NEG / (1.0 - NEG)))
        nc.vector.tensor_tensor(out=res[:], in0=t1[:], in1=sel[:], op=ALU.add)
    return res


def _split_multi_waits(nc):
    """TRN2 compute/DMA instructions encode at most one semaphore wait.
    Tile sometimes emits several; hoist the extras onto standalone
    EventSemaphore instructions placed just before (same engine stream)."""
    n = 0
    for f in nc.m.functions:
        for blk in f.blocks:
            out = []
            changed = False
            for inst in blk.instructions:
                si = inst.sync_info
                if si is not None and si.on_wait and len(si.on_wait) > 1:
                    waits = list(si.on_wait)
                    for w in waits[:-1]:
                        ev = mybir.InstEventSemaphore(
                            name=f"sw-{n}-{inst.name}", engine=inst.engine,
                            ins=[], outs=[],
                            sync_info=mybir.SyncInfo(on_wait=[w],
                                                     on_update=[]))
                        out.append(ev)
                        n += 1
                    inst.sync_info = mybir.SyncInfo(
                        on_wait=[waits[-1]], on_update=list(si.on_update))
                    changed = True
                out.append(inst)
            if changed:
                blk.instructions = out
    return n


def _build_bass(split_waits=True):
    nc = bass.Bass()

    sp = nc.declare_dram_parameter("sp", [N2, 4], F32, isOutput=False)
    qa = nc.declare_dram_parameter("qa", [TP, NT * 3], F32, isOutput=False)
    ia = nc.declare_dram_parameter("ia", [TP, NT * K], I32, isOutput=False)
    w_shapes = dict(wvn=48, wdv=48, wh1=128, wh2=64, bh2=8, wbr=768,
                    wdr=1024, wun=2048, wdu=2048)
    wparams = {k: nc.declare_dram_parameter(k, [1, v], F32, isOutput=False)
               for k, v in w_shapes.items()}
    out = nc.declare_dram_parameter("out", [NLOC, OUT * 3], F32, isOutput=True)

    with tile.TileContext(nc) as tc:
        with (
            tc.tile_pool(name="wts", bufs=1) as wpool,
            tc.tile_pool(name="g", bufs=NT) as gpool,
            tc.tile_pool(name="work", bufs=2) as pool,
            tc.tile_pool(name="jumbo", bufs=1) as jpool,
        ):
            wsb = {}
            for k_, v in w_shapes.items():
                wsb[k_] = wpool.tile([128, v], F32, tag=f"w_{k_}",
                                     name=f"w_{k_}")
                nc.sync.dma_start(
                    out=wsb[k_][:],
                    in_=wparams[k_][:, :].to_broadcast([128, v]))
            qsb = wpool.tile([TP, NT * 3], F32, tag="qsb")
            nc.sync.dma_start(out=qsb[:], in_=qa[:, :])
            isb = wpool.tile([TP, NT * K], I32, tag="isb")
            nc.sync.dma_start(out=isb[:], in_=ia[:, :])

            # "touch" every staged tile once per consumer engine so later
            # instructions never need more than one DMA-sem wait each
            # (TRN2 allows a single sync-wait per compute instruction).
            dummy = wpool.tile([128, 16], F32, tag="dummy")
            dummyi = wpool.tile([128, 2], I32, tag="dummyi")
            for i, k_ in enumerate(w_shapes):
                nc.vector.tensor_scalar(out=dummy[:, i:i + 1],
                                        in0=wsb[k_][:, :1], scalar1=1.0,
                                        scalar2=None, op0=ALU.mult)
            nc.vector.tensor_scalar(out=dummy[:TP, 10:11], in0=qsb[:, :1],
                                    scalar1=1.0, scalar2=None, op0=ALU.mult)
            nc.gpsimd.tensor_scalar(out=dummyi[:TP, :1], in0=isb[:, :1],
                                    scalar1=1, scalar2=None, op0=ALU.mult)

            for t in range(NT):
                # ---- gather neighbors: G[p, k*3+d] = s_pts[idx[p,k], d]
                # HW indirect DMA uses one index per partition, so issue one
                # DMA per neighbor slot k.
                G = gpool.tile([TP, K * 4], F32, tag="G")
                for k in range(K):
                    nc.gpsimd.indirect_dma_start(
                        out=G[:, k * 4:(k + 1) * 4], out_offset=None,
                        in_=sp[:, :],
                        in_offset=bass.IndirectOffsetOnAxis(
                            ap=isb[:, t * K + k:t * K + k + 1], axis=0))

                # ---- pts = G - q  (broadcast q over k)
                pts = pool.tile([TP, K * 3], F32, tag="pts")
                q_b = (qsb[:, t * 3:(t + 1) * 3].unsqueeze(1)
                       .to_broadcast([TP, K, 3]))
                nc.vector.tensor_tensor(
                    out=pts[:].rearrange("p (k d) -> p k d", k=K, d=3),
                    in0=G[:].rearrange("p (k e) -> p k e", k=K, e=4)[:, :, :3],
                    in1=q_b, op=ALU.subtract)

                def pts_kd():
                    return pts[:].rearrange("p (k d) -> p k d", k=K, d=3)

                def pts_dk():
                    return pts[:].rearrange("p (k d) -> p d k", k=K, d=3)

                # ---- centers = mean_k pts  [TP, 3]
                cen = pool.tile([TP, 3], F32, tag="cen")
                nc.vector.tensor_reduce(out=cen[:], in_=pts_dk(),
                                        axis=AXIS.X, op=ALU.add)
                nc.vector.tensor_scalar(out=cen[:], in0=cen[:],
                                        scalar1=1.0 / K, scalar2=None,
                                        op0=ALU.mult)

                # ---- cross = pts x cen  [TP, (k,d)]
                cross = pool.tile([TP, K * 3], F32, tag="cross")
                crv = cross[:].rearrange("p (k d) -> p k d", k=K, d=3)
                ct1 = pool.tile([TP, K], F32, tag="ct1")
                ct2 = pool.tile([TP, K], F32, tag="ct2")
                for dd in range(3):
                    d1, d2 = (dd + 1) % 3, (dd + 2) % 3
                    nc.vector.tensor_tensor(
                        out=ct1[:], in0=pts_kd()[:, :, d1],
                        in1=cen[:, d2:d2 + 1].to_broadcast([TP, K]),
                        op=ALU.mult)
                    nc.vector.tensor_tensor(
                        out=ct2[:], in0=pts_kd()[:, :, d2],
                        in1=cen[:, d1:d1 + 1].to_broadcast([TP, K]),
                        op=ALU.mult)
                    nc.vector.tensor_tensor(
                        out=crv[:, :, dd], in0=ct1[:], in1=ct2[:],
                        op=ALU.subtract)

                if DBG == "local":
                    nc.sync.dma_start(out=out[t * TP:(t + 1) * TP, :48],
                                      in_=pts[:])
                    nc.sync.dma_start(out=out[t * TP:(t + 1) * TP, 48:96],
                                      in_=cross[:])
                    nc.sync.dma_start(out=out[t * TP:(t + 1) * TP, 96:99],
                                      in_=cen[:])
                    nc.sync.dma_start(
                        out=out[t * TP:(t + 1) * TP, 99:147]
                        .rearrange("p (k d) -> p k d", k=K, d=3),
                        in_=G[:].rearrange("p (k e) -> p k e", k=K, e=4)
                        [:, :, :3])
                    continue

                def cross_dk():
                    return cross[:].rearrange("p (k d) -> p d k", k=K, d=3)

                def cen_b(o):  # [TP, o, 3, K] broadcast
                    return (cen[:].unsqueeze(1).unsqueeze(3)
                            .to_broadcast([TP, o, 3, K]))

                def loc_b(ap, o):  # local (k,d) tile -> [TP, o, 3, K]
                    return (ap.rearrange("p (k d) -> p d k", k=K, d=3)
                            .unsqueeze(1).to_broadcast([TP, o, 3, K]))

                # ---- p = vn_lin(local, wvn) (BN folded), layout (o, d, k)
                def emit_vnlin(wtile, o, dst_tag):
                    dst = pool.tile([TP, o * 3 * K], F32, tag=dst_tag)
                    dv = dst[:].rearrange("p (o d k) -> p o d k", o=o, d=3, k=K)
                    wv = wtile[:TP, :].rearrange("p (c o) -> p c o", c=3, o=o)
                    tmp = pool.tile([TP, o * 3 * K], F32, tag="vl_tmp", bufs=2)
                    tv = tmp[:].rearrange("p (o d k) -> p o d k", o=o, d=3, k=K)
                    for c, src in enumerate((loc_b(pts[:], o), cen_b(o),
                                             loc_b(cross[:], o))):
                        wb_ = (wv[:, c].unsqueeze(2).unsqueeze(3)
                               .to_broadcast([TP, o, 3, K]))
                        if c == 0:
                            nc.vector.tensor_tensor(out=dv, in0=src, in1=wb_,
                                                    op=ALU.mult)
                        else:
                            nc.vector.tensor_tensor(out=tv, in0=src, in1=wb_,
                                                    op=ALU.mult)
                            nc.vector.tensor_tensor(out=dv, in0=dv, in1=tv,
                                                    op=ALU.add)
                    return dst

                P_ = emit_vnlin(wsb["wvn"], 2 * KS, "P_")
                D_ = emit_vnlin(wsb["wdv"], 2 * KS, "D_")

                # ---- s = vn_act(p, d) with 0.8 folded into wh1
                S_ = _emit_vn_act(nc, pool, P_[:], D_[:], 2 * KS, K,
                                  out_scale_blend=False, tag="va1")

                # ---- snorm = ||s||_d  [TP, (o,k)]
                ss = pool.tile([TP, 2 * KS * 3 * K], F32, tag="va1_pd",
                               bufs=2)
                nc.vector.tensor_tensor(out=ss[:], in0=S_[:], in1=S_[:],
                                        op=ALU.mult)
                nsq = pool.tile([TP, 2 * KS * K], F32, tag="nsq")
                nc.vector.tensor_reduce(
                    out=nsq[:].rearrange("p (o k) -> p o k", o=2 * KS, k=K),
                    in_=ss[:].rearrange("p (o d k) -> p o k d",
                                        o=2 * KS, d=3, k=K),
                    axis=AXIS.X, op=ALU.add)
                sn = pool.tile([TP, 2 * KS * K], F32, tag="sn")
                nc.scalar.activation(out=sn[:], in_=nsq[:], func=ACTF.Sqrt)

                # ---- h1: z1[o2,k] = relu(sum_o sn[o,k]*wh1[o,o2])
                tw = jpool.tile([TP, 3072], F32, tag="jumbo")
                twv = (tw[:, :KS * K * 2 * KS]
                       .rearrange("p (o2 k o) -> p o2 k o",
                                  o2=KS, k=K, o=2 * KS))
                sn_b = (sn[:].rearrange("p (o k) -> p k o", o=2 * KS, k=K)
                        .unsqueeze(1).to_broadcast([TP, KS, K, 2 * KS]))
                wh1_b = (wsb["wh1"][:TP, :]
                         .rearrange("p (o o2) -> p o2 o", o=2 * KS, o2=KS)
                         .unsqueeze(2).to_broadcast([TP, KS, K, 2 * KS]))
                nc.vector.tensor_tensor(out=twv, in0=sn_b, in1=wh1_b,
                                        op=ALU.mult)
                z1 = pool.tile([TP, KS * K], F32, tag="z1")
                nc.vector.tensor_reduce(
                    out=z1[:].rearrange("p (o2 k) -> p o2 k", o2=KS, k=K),
                    in_=twv, axis=AXIS.X, op=ALU.add)
                z1r = pool.tile([TP, KS * K], F32, tag="z1r")
                nc.scalar.activation(out=z1r[:], in_=z1[:], func=ACTF.Relu)

                # ---- h2 + bias
                tw2v = (tw[:, :KS * K * KS]
                        .rearrange("p (o3 k o2) -> p o3 k o2",
                                   o3=KS, k=K, o2=KS))
                z1_b = (z1r[:].rearrange("p (o2 k) -> p k o2", o2=KS, k=K)
                        .unsqueeze(1).to_broadcast([TP, KS, K, KS]))
                wh2_b = (wsb["wh2"][:TP, :]
                         .rearrange("p (o2 o3) -> p o3 o2", o2=KS, o3=KS)
                         .unsqueeze(2).to_broadcast([TP, KS, K, KS]))
                nc.vector.tensor_tensor(out=tw2v, in0=z1_b, in1=wh2_b,
                                        op=ALU.mult)
                z2 = pool.tile([TP, KS * K], F32, tag="z2")
                nc.vector.tensor_reduce(
                    out=z2[:].rearrange("p (o3 k) -> p o3 k", o3=KS, k=K),
                    in_=tw2v, axis=AXIS.X, op=ALU.add)
                bh2_b = (wsb["bh2"][:TP, :].unsqueeze(2)
                         .to_broadcast([TP, KS, K]))
                z2v = z2[:].rearrange("p (o3 k) -> p o3 k", o3=KS, k=K)
                nc.vector.tensor_tensor(out=z2v, in0=z2v, in1=bh2_b,
                                        op=ALU.add)

                # ---- softmax over o3
                mx = pool.tile([TP, K], F32, tag="mx")
                nc.vector.tensor_reduce(
                    out=mx[:],
                    in_=z2[:].rearrange("p (o3 k) -> p k o3", o3=KS, k=K),
                    axis=AXIS.X, op=ALU.max)
                mx_b = mx[:].unsqueeze(1).to_broadcast([TP, KS, K])
                ex = pool.tile([TP, KS * K], F32, tag="ex")
                exv = ex[:].rearrange("p (o3 k) -> p o3 k", o3=KS, k=K)
                nc.vector.tensor_tensor(out=exv, in0=z2v, in1=mx_b,
                                        op=ALU.subtract)
                exe = pool.tile([TP, KS * K], F32, tag="exe")
                nc.scalar.activation(out=exe[:], in_=ex[:], func=ACTF.Exp)
                exv = exe[:].rearrange("p (o3 k) -> p o3 k", o3=KS, k=K)
                sme = pool.tile([TP, K], F32, tag="sme")
                nc.vector.tensor_reduce(
                    out=sme[:],
                    in_=exe[:].rearrange("p (o3 k) -> p k o3", o3=KS, k=K),
                    axis=AXIS.X, op=ALU.add)
                rcs = pool.tile([TP, K], F32, tag="rcs")
                nc.vector.reciprocal(out=rcs[:], in_=sme[:])
                sc = pool.tile([TP, KS * K], F32, tag="sc")
                scv = sc[:].rearrange("p (ks k) -> p ks k", ks=KS, k=K)
                nc.vector.tensor_tensor(
                    out=scv, in0=exv,
                    in1=rcs[:].unsqueeze(1).to_broadcast([TP, KS, K]),
                    op=ALU.mult)

                # ---- weff[(c,h),k] = sum_ks sc[ks,k] * wbr[(c,h),ks]
                weff = pool.tile([TP, 96 * K], F32, tag="weff", bufs=2)
                KC = K // 4
                for kc in range(4):
                    tw3v = (tw[:, :96 * KC * KS]
                            .rearrange("p (ch k ks) -> p ch k ks",
                                       ch=96, k=KC, ks=KS))
                    sc_b = (sc[:].rearrange("p (ks k) -> p k ks", ks=KS, k=K)
                            [:, kc * KC:(kc + 1) * KC]
                            .unsqueeze(1).to_broadcast([TP, 96, KC, KS]))
                    wbr_b = (wsb["wbr"][:TP, :]
                             .rearrange("p (ch ks) -> p ch ks", ch=96, ks=KS)
                             .unsqueeze(2).to_broadcast([TP, 96, KC, KS]))
                    nc.gpsimd.tensor_tensor(out=tw3v, in0=sc_b,
                                            in1=wbr_b, op=ALU.mult)
                    nc.vector.tensor_reduce(
                        out=weff[:].rearrange("p (ch k) -> p ch k",
                                              ch=96, k=K)
                        [:, :, kc * KC:(kc + 1) * KC],
                        in_=tw3v, axis=AXIS.X, op=ALU.add)

                # ---- pro[(h,d),k] = sum_c weff[(c,h),k] * local[c][d,k]
                pro = pool.tile([TP, HALF * 3 * K], F32, tag="pro", bufs=2)
                prov = pro[:].rearrange("p (h d k) -> p h d k",
                                        h=HALF, d=3, k=K)
                wev = weff[:].rearrange("p (c h k) -> p c h k",
                                        c=3, h=HALF, k=K)
                ptmp = pool.tile([TP, HALF * 3 * K], F32, tag="ptmp", bufs=2)
                ptv = ptmp[:].rearrange("p (h d k) -> p h d k",
                                        h=HALF, d=3, k=K)
                for c, src in enumerate((loc_b(pts[:], HALF), cen_b(HALF),
                                         loc_b(cross[:], HALF))):
                    we_b = wev[:, c].unsqueeze(2).to_broadcast(
                        [TP, HALF, 3, K])
                    if c == 0:
                        nc.vector.tensor_tensor(out=prov, in0=we_b, in1=src,
                                                op=ALU.mult)
                    else:
                        nc.vector.tensor_tensor(out=ptv, in0=we_b, in1=src,
                                                op=ALU.mult)
                        nc.vector.tensor_tensor(out=prov, in0=prov, in1=ptv,
                                                op=ALU.add)

                # ---- normalize over d, then mean over k -> feats [TP, (h,d)]
                q2 = pool.tile([TP, HALF * 3 * K], F32, tag="ptmp", bufs=2)
                nc.vector.tensor_tensor(out=q2[:], in0=pro[:], in1=pro[:],
                                        op=ALU.mult)
                ns = pool.tile([TP, HALF * K], F32, tag="ns")
                nc.vector.tensor_reduce(
                    out=ns[:].rearrange("p (h k) -> p h k", h=HALF, k=K),
                    in_=q2[:].rearrange("p (h d k) -> p h k d",
                                        h=HALF, d=3, k=K),
                    axis=AXIS.X, op=ALU.add)
                nr = pool.tile([TP, HALF * K], F32, tag="nr")
                nc.scalar.activation(out=nr[:], in_=ns[:], func=ACTF.Sqrt)
                nrc = pool.tile([TP, HALF * K], F32, tag="nrc")
                nc.vector.tensor_scalar(out=nrc[:], in0=nr[:], scalar1=1e-12,
                                        scalar2=None, op0=ALU.max)
                rcn = pool.tile([TP, HALF * K], F32, tag="rcn")
                nc.vector.reciprocal(out=rcn[:], in_=nrc[:])
                nmd = pool.tile([TP, HALF * 3 * K], F32, tag="nmd", bufs=2)
                rcn_b = (rcn[:].rearrange("p (h k) -> p h k", h=HALF, k=K)
                         .unsqueeze(2).to_broadcast([TP, HALF, 3, K]))
                nc.vector.tensor_tensor(
                    out=nmd[:].rearrange("p (h d k) -> p h d k",
                                         h=HALF, d=3, k=K),
                    in0=prov, in1=rcn_b, op=ALU.mult)
                fe = pool.tile([TP, HALF * 3], F32, tag="fe")
                nc.vector.tensor_reduce(
                    out=fe[:].rearrange("p (h d) -> p h d", h=HALF, d=3),
                    in_=nmd[:].rearrange("p (h d k) -> p h d k",
                                         h=HALF, d=3, k=K),
                    axis=AXIS.X, op=ALU.add)
                nc.vector.tensor_scalar(out=fe[:], in0=fe[:], scalar1=1.0 / K,
                                        scalar2=None, op0=ALU.mult)

                # ---- d1 = vn_lin(fe, wd_relu); f2 = vn_act(fe, d1)
                def emit_fc(src_tile, wtile, ci, co, dst_tag):
                    # dst[(o,d)] = sum_hi src[(hi,d)] * w[(o,hi)]
                    dst = pool.tile([TP, co * 3], F32, tag=dst_tag)
                    OC = min(co, 3072 // (3 * ci))
                    src_b = (src_tile[:].rearrange("p (h d) -> p d h",
                                                   h=ci, d=3)
                             .unsqueeze(1).to_broadcast([TP, OC, 3, ci]))
                    for oc in range(0, co, OC):
                        twv_ = (tw[:, :OC * 3 * ci]
                                .rearrange("p (o d hi) -> p o d hi",
                                           o=OC, d=3, hi=ci))
                        w_b = (wtile[:TP, oc * ci:(oc + OC) * ci]
                               .rearrange("p (o hi) -> p o hi", o=OC, hi=ci)
                               .unsqueeze(2).to_broadcast([TP, OC, 3, ci]))
                        nc.vector.tensor_tensor(out=twv_, in0=src_b, in1=w_b,
                                                op=ALU.mult)
                        nc.vector.tensor_reduce(
                            out=dst[:].rearrange("p (o d) -> p o d",
                                                 o=co, d=3)
                            [:, oc:oc + OC],
                            in_=twv_, axis=AXIS.X, op=ALU.add)
                    return dst

                d1 = emit_fc(fe, wsb["wdr"], HALF, HALF, "d1")
                f2 = _emit_vn_act(nc, pool, fe[:], d1[:], HALF, 1,
                                  out_scale_blend=True, tag="va2")

                # ---- out = vn_act(vn_lin(f2, w_un)*BN, vn_lin(f2, wd_un))
                u = emit_fc(f2, wsb["wun"], HALF, OUT, "u")
                du = emit_fc(f2, wsb["wdu"], HALF, OUT, "du")
                ot = _emit_vn_act(nc, pool, u[:], du[:], OUT, 1,
                                  out_scale_blend=True, tag="va3")

                nc.sync.dma_start(out=out[t * TP:(t + 1) * TP, :], in_=ot[:])

    if split_waits:
        _split_multi_waits(nc)
    return nc


def _prep_core_inputs(core, q_pts, s_pts, idx, weights):
    q = q_pts[core * NLOC:(core + 1) * NLOC].astype(np.float32)
    ix = idx[core * NLOC:(core + 1) * NLOC].astype(np.int32)
    qa = np.ascontiguousarray(
        q.reshape(NT, TP, 3).transpose(1, 0, 2).reshape(TP, NT * 3))
    ia = np.ascontiguousarray(
        ix.reshape(NT, TP, K).transpose(1, 0, 2).reshape(TP, NT * K))
    sp4 = np.zeros((s_pts.shape[0], 4), np.float32)
    sp4[:, :3] = s_pts
    m = {"sp": sp4, "qa": qa, "ia": ia}
    m.update(weights)
    return m


def kernel(q_pts, s_pts, s_feats, neighbor_indices, wb, w_vn, wd_vn,
           w_h1, w_h2, b_h2, wd_relu, w_un, wd_un):
    q_pts = np.asarray(q_pts, dtype=np.float32)
    s_pts = np.asarray(s_pts, dtype=np.float32)
    idx = np.asarray(neighbor_indices)
    wb = np.asarray(wb, np.float32); w_vn = np.asarray(w_vn, np.float32)
    wd_vn = np.asarray(wd_vn, np.float32); w_h1 = np.asarray(w_h1, np.float32)
    w_h2 = np.asarray(w_h2, np.float32); b_h2 = np.asarray(b_h2, np.float32)
    wd_relu = np.asarray(wd_relu, np.float32)
    w_un = np.asarray(w_un, np.float32); wd_un = np.asarray(wd_un, np.float32)

    def rep(a):
        return np.ascontiguousarray(np.asarray(a).reshape(1, -1),
                                    dtype=np.float32)

    # host-side weight packing (layouts documented at each use site)
    weights = {
        "wvn": rep((w_vn * BN)),                                  # (c,o)
        "wdv": rep(wd_vn),                                        # (c,o)
        "wh1": rep(w_h1 * BN * (1.0 - NEG)),                      # (o,o2)
        "wh2": rep(w_h2),                                         # (o2,o3)
        "bh2": rep(b_h2),                                         # (o3,)
        # wbr[(c,h),ks] = wb[c, ks*HALF+h]
        "wbr": rep(wb.reshape(3, KS, HALF).transpose(0, 2, 1)),   # (c,h,ks)
        # wdr[(ho,hi)] = wd_relu[hi,ho]
        "wdr": rep(wd_relu.T),
        "wun": rep((w_un * BN).T),                                # (o,hi)
        "wdu": rep(wd_un.T),                                      # (o,hi)
    }

    if "nc" not in _CACHE:
        _CACHE["nc"] = _build_bass()
    nc = _CACHE["nc"]

    in_maps = [_prep_core_inputs(c, q_pts, s_pts, idx, weights)
               for c in range(NCORES)]
    res = run_bass_kernel_spmd(nc, in_maps, core_ids=list(range(NCORES)))
    outs = [np.asarray(r["out"]).reshape(NLOC, OUT, 3) for r in res.results]
    return np.concatenate(outs, axis=0)
